# revision 1
# baseline (speedup 1.0000x reference)
"""Tensor-parallel Llama layer on 8 Trainium2 NeuronCores (Bass/Tile).

Sharding: TP per the hint — wq/wk/wv/wg/wh column-sharded (4 q-heads + 1 kv
head + 1792 ffn rows per core), wo/wf row-sharded with ReduceScatter after
attention-out and ffn-out; sequence-parallel RMSNorms (256 tokens/core) with
AllGather of the normed activations (bf16).

Activations are kept feature-major (x.T layout) on chip so every projection
is a plain lhsT.T @ rhs with contraction on the partition axis. Weights are
pre-transposed and pre-cast to bf16 on the host (host prep is free).
"""
import sys

sys.path.insert(0, '/opt/trn_rl_repo')
from contextlib import ExitStack

import numpy as np
import ml_dtypes

import concourse.bass as bass
import concourse.tile as tile
from concourse import bacc, mybir
from concourse.bass_utils import run_bass_kernel_spmd

AF = mybir.ActivationFunctionType
ALU = mybir.AluOpType
BF16 = mybir.dt.bfloat16
F32 = mybir.dt.float32

CORES = 8
DH = 128
EPS = 1e-5
TBLK = 512
NEG_BIG = -1e30

FULL_CFG = dict(N=2048, D=4096, QH=4, FC=1792)

# CoreSim doesn't implement Silu; set True to build with Sigmoid + an extra
# multiply (same math) for simulator validation.
SILU_VIA_SIGMOID = False

# ReduceScatter the attention/ffn partial sums in bf16 (halves collective
# time); flip to False if accuracy needs the headroom.
RS_BF16 = True

# Feature-chunks per collective (pipelines collectives behind compute).
NCH = 4


def build_module(cfg):
    N, D, QH, FC = cfg['N'], cfg['D'], cfg['QH'], cfg['FC']
    C = CORES
    NB = N // C            # tokens per core block
    TT = NB // 128         # token tiles per core block
    KP = D // 128          # d_model contraction chunks
    NBLK = N // TBLK       # matmul token blocks
    BPT = TBLK // NB       # DRAM token-blocks per matmul token block
    KCH = N // DH          # attention k chunks
    QT = N // TBLK         # q tiles per head
    FM = FC // DH          # ffn M tiles per core
    T2 = N // 2            # ffn token half
    NS2 = T2 // TBLK       # 512-subblocks per ffn half
    BPH = C // 2           # DRAM token-blocks per ffn half
    BPS = TBLK // NB       # DRAM token-blocks per 512-subblock
    MQKV = QH + 2
    scale = float(1.0 / np.sqrt(DH))

    nc = bacc.Bacc("TRN2", target_bir_lowering=False, debug=False, num_devices=C)

    x_c = nc.dram_tensor("x_c", [NB, D], F32, kind="ExternalInput")
    wqT = nc.dram_tensor("wqT", [D, QH * DH], BF16, kind="ExternalInput")
    wkT = nc.dram_tensor("wkT", [D, DH], BF16, kind="ExternalInput")
    wvT = nc.dram_tensor("wvT", [D, DH], BF16, kind="ExternalInput")
    woT = nc.dram_tensor("woT", [QH * DH, D], BF16, kind="ExternalInput")
    wgT = nc.dram_tensor("wgT", [D, FC], BF16, kind="ExternalInput")
    whT = nc.dram_tensor("whT", [D, FC], BF16, kind="ExternalInput")
    wfT = nc.dram_tensor("wfT", [FC, D], BF16, kind="ExternalInput")
    rcosT = nc.dram_tensor("rcosT", [DH, N], F32, kind="ExternalInput")
    rsinT = nc.dram_tensor("rsinT", [DH, N], F32, kind="ExternalInput")
    swapT = nc.dram_tensor("swapT", [DH, DH], BF16, kind="ExternalInput")
    diagneg = nc.dram_tensor("diagneg", [DH, DH], BF16, kind="ExternalInput")
    identb = nc.dram_tensor("identb", [128, 128], BF16, kind="ExternalInput")
    identf = nc.dram_tensor("identf", [128, 128], F32, kind="ExternalInput")
    onesc = nc.dram_tensor("onesc", [128, 128], BF16, kind="ExternalInput")
    masks = nc.dram_tensor("masks", [4, 128, TBLK], BF16, kind="ExternalInput")
    out_c = nc.dram_tensor("out_c", [NB, D], F32, kind="ExternalOutput")

    RSDT = BF16 if RS_BF16 else F32
    nch = max(1, min(NCH, D // 512))   # effective chunk count
    DCH = D // nch          # features per collective chunk
    KPC = KP // nch         # kp (128-feature) tiles per chunk
    assert KP % nch == 0 and DCH % 512 == 0
    even = [KPC * i for i in range(nch + 1)]
    # AllGathers: fewer chunks (each ~15us collective floor dominates small
    # chunks); ReduceScatters keep 4-way chunking for producer overlap.
    nag = max(1, min(2, nch))
    AG_CUTS = [KP // nag * i for i in range(nag + 1)]
    RS1_CUTS = RS2_CUTS = even

    def ch_of(cuts, kp):
        for c in range(len(cuts) - 1):
            if kp < cuts[c + 1]:
                return c, kp - cuts[c]
        raise ValueError

    with tile.TileContext(nc) as tc, ExitStack() as top:
        dram = top.enter_context(tc.tile_pool(name="dram", bufs=1, space="DRAM"))

        def dram_chunks(nm, cuts, mul, dt, shared=False):
            kw = dict(addr_space="Shared") if shared else {}
            return [dram.tile([(cuts[i + 1] - cuts[i]) * 128 * mul, NB], dt,
                              tag=f"{nm}{i}", name=f"{nm}{i}", **kw)
                    for i in range(len(cuts) - 1)]

        r2d = dram.tile([NB, D], F32, tag="r2d", name="r2d")
        hT_in_ch = dram_chunks("hT_in", AG_CUTS, 1, BF16)
        hT_all_ch = dram_chunks("hT_all", AG_CUTS, C, BF16, shared=True)
        opart_ch = dram_chunks("opart", RS1_CUTS, C, RSDT)
        ored_ch = dram_chunks("ored", RS1_CUTS, 1, RSDT)
        h2T_in_ch = dram_chunks("h2T_in", AG_CUTS, 1, BF16)
        h2T_all_ch = dram_chunks("h2T_all", AG_CUTS, C, BF16, shared=True)
        fpart_ch = dram_chunks("fpart", RS2_CUTS, C, RSDT)
        fred_ch = dram_chunks("fred", RS2_CUTS, 1, RSDT)

        # ---- constants resident in SBUF ----
        const = top.enter_context(tc.tile_pool(name="const", bufs=1))
        swap_sb = const.tile([DH, DH], BF16, tag="swap", name="swap")
        diag_sb = const.tile([DH, DH], BF16, tag="diag", name="diag")
        identb_sb = const.tile([128, 128], BF16, tag="identb", name="identb")
        identf_sb = const.tile([128, 128], F32, tag="identf", name="identf")
        ones_sb = const.tile([128, 128], BF16, tag="ones", name="ones")
        masks_sb = const.tile([128, 4 * TBLK], BF16, tag="masks", name="masks")
        nc.sync.dma_start(swap_sb[:], swapT.ap())
        nc.sync.dma_start(diag_sb[:], diagneg.ap())
        nc.sync.dma_start(identb_sb[:], identb.ap())
        nc.sync.dma_start(identf_sb[:], identf.ap())
        nc.sync.dma_start(ones_sb[:], onesc.ap())
        nc.sync.dma_start(
            masks_sb[:].rearrange("p (r t) -> p r t", r=4),
            masks.ap().rearrange("r p t -> p r t"),
        )

        # ---- shared PSUM pools (total 4+3+1 = 8 banks) ----
        ps_acc = top.enter_context(tc.tile_pool(name="ps_acc", bufs=4, space="PSUM"))
        ps_tmp = top.enter_context(tc.tile_pool(name="ps_tmp", bufs=3, space="PSUM"))
        ps_sml = top.enter_context(tc.tile_pool(name="ps_sml", bufs=1, space="PSUM"))

        # ---- attention residents (freed after P3; opened last for LIFO) ----
        attn_ctx = ExitStack()
        attn = attn_ctx.enter_context(tc.tile_pool(name="attn", bufs=1))
        rcos_sb = attn.tile([DH, N], F32, tag="rcos", name="rcos")
        rsin_sb = attn.tile([DH, N], F32, tag="rsin", name="rsin")
        nc.sync.dma_start(rcos_sb[:], rcosT.ap())
        nc.sync.dma_start(rsin_sb[:], rsinT.ap())
        qrot = [attn.tile([DH, N], BF16, tag=f"qrot{h}", name=f"qrot{h}") for h in range(QH)]
        krot = attn.tile([DH, N], BF16, tag="krot", name="krot")
        vsb = attn.tile([DH, N], BF16, tag="vsb", name="vsb")
        vtok = attn.tile([128, KCH * DH], BF16, tag="vtok", name="vtok")
        aT = [attn.tile([DH, N], BF16, tag=f"aT{h}", name=f"aT{h}") for h in range(QH)]

        def seqpar_norm_and_gather(src_tiles, dst_chunks, out_chunks, pool,
                                   pspool, prefix):
            """src_tiles: TT SBUF tiles [128, D] f32 (token-major rows of this
            core's block). RMS-normalize each row, transpose to feature-major
            chunk tensors [DCH, NB], then AllGather each chunk."""
            for t in range(TT):
                xt = src_tiles[t]
                sq = pool.tile([128, D], F32, tag=f"{prefix}sq", name=f"{prefix}sq")
                ssum = pool.tile([128, 1], F32, tag=f"{prefix}ss", name=f"{prefix}ss")
                nc.scalar.activation(sq[:], xt[:], AF.Square, accum_out=ssum[:])
                var = pool.tile([128, 1], F32, tag=f"{prefix}var", name=f"{prefix}var")
                nc.vector.tensor_scalar(
                    out=var[:], in0=ssum[:], scalar1=1.0 / D, scalar2=EPS,
                    op0=ALU.mult, op1=ALU.add)
                sv = pool.tile([128, 1], F32, tag=f"{prefix}sv", name=f"{prefix}sv")
                nc.scalar.activation(sv[:], var[:], AF.Sqrt)
                rstd = pool.tile([128, 1], F32, tag=f"{prefix}rstd", name=f"{prefix}rstd")
                nc.vector.reciprocal(rstd[:], sv[:])
                htok = pool.tile([128, D], BF16, tag=f"{prefix}h", name=f"{prefix}h")
                nc.vector.tensor_scalar_mul(htok[:], xt[:], rstd[:])
                for g in range(D // 512):
                    ps = pspool.tile([128, 512], BF16, tag="tmp", name="tps")
                    for q4 in range(4):
                        dd = 4 * g + q4
                        nc.tensor.transpose(
                            ps[:, 128 * q4:128 * (q4 + 1)],
                            htok[:, 128 * dd:128 * (dd + 1)], identb_sb[:])
                    ev = pool.tile([128, 512], BF16, tag=f"{prefix}ev", name=f"{prefix}ev")
                    if g % 2 == 0:
                        nc.vector.tensor_copy(ev[:], ps[:])
                    else:
                        nc.scalar.activation(ev[:], ps[:], AF.Copy)
                    ch, kpl0 = ch_of(AG_CUTS, 4 * g)
                    r0 = 128 * kpl0
                    nc.gpsimd.dma_start(
                        dst_chunks[ch][r0:r0 + 512, 128 * t:128 * (t + 1)]
                        .rearrange("(q d) t -> d q t", q=4),
                        ev[:].rearrange("p (q t) -> p q t", q=4))
            for ch in range(len(AG_CUTS) - 1):
                nc.gpsimd.collective_compute(
                    "AllGather", ALU.bypass, replica_groups=[list(range(C))],
                    ins=[dst_chunks[ch][:].opt()], outs=[out_chunks[ch][:].opt()])

        # ================= P0: norm1 (seq-parallel) + chunked AllGather ====
        with ExitStack() as ctx:
            p0 = ctx.enter_context(tc.tile_pool(name="p0", bufs=1))
            x_tiles = []
            for t in range(TT):
                xt = p0.tile([128, D], F32, tag=f"x{t}", name=f"x{t}")
                nc.sync.dma_start(xt[:], x_c.ap()[128 * t:128 * (t + 1), :])
                x_tiles.append(xt)
            seqpar_norm_and_gather(x_tiles, hT_in_ch, hT_all_ch, p0, ps_tmp, "n1")

        hT_views = [hT_all_ch[ch][:].rearrange("(b d) t -> d b t", b=C)
                    for ch in range(len(AG_CUTS) - 1)]

        # ================= P1: QKV + RoPE (per token block) =================
        with ExitStack() as ctx:
            wsl = ctx.enter_context(tc.tile_pool(name="qkv_w", bufs=1))
            rhsp = ctx.enter_context(tc.tile_pool(name="qkv_rhs", bufs=1))
            ep = ctx.enter_context(tc.tile_pool(name="qkv_ep", bufs=3))
            # QKV weight slabs are small (6 x 8KB/partition bf16): load once
            slabs = []
            for m in range(MQKV):
                slab = wsl.tile([128, KP * 128], BF16, tag=f"w{m}", name=f"w{m}")
                if m < QH:
                    src = wqT.ap()[:, 128 * m:128 * (m + 1)]
                elif m == QH:
                    src = wkT.ap()
                else:
                    src = wvT.ap()
                nc.sync.dma_start(
                    slab[:].rearrange("p (k m) -> p k m", m=128),
                    src.rearrange("(k p) m -> p k m", p=128))
                slabs.append(slab)

            def rope(dst, src_sb, ps_swap, sl):
                """dst[:, sl] = src*cos + (P@src)*sin; src_sb bf16, ps_swap psum."""
                t1 = ep.tile([128, TBLK], F32, tag="rope_t1", name="rope_t1")
                nc.vector.tensor_tensor(t1[:], src_sb[:], rcos_sb[:, sl], op=ALU.mult)
                t2 = ep.tile([128, TBLK], F32, tag="rope_t2", name="rope_t2")
                nc.vector.tensor_tensor(t2[:], ps_swap[:], rsin_sb[:, sl], op=ALU.mult)
                nc.vector.tensor_tensor(dst[:, sl], t1[:], t2[:], op=ALU.add)

            for nb in range(NBLK):
                sl = slice(TBLK * nb, TBLK * (nb + 1))
                # one rhs load per (nb, kp), shared by both M-groups
                rtiles = []
                for kp in range(KP):
                    rt = rhsp.tile([128, TBLK], BF16, tag=f"rhs{kp}", name=f"rhs{kp}")
                    chq, kpl = ch_of(AG_CUTS, kp)
                    nc.sync.dma_start(
                        rt[:].rearrange("p (b t) -> p b t", b=BPT),
                        hT_views[chq][128 * kpl:128 * (kpl + 1),
                                      BPT * nb:BPT * (nb + 1), :])
                    rtiles.append(rt)
                for hm in range(2):
                    group = list(range(3 * hm, min(3 * (hm + 1), MQKV)))
                    gacc = {m: ps_acc.tile([128, TBLK], F32, tag="acc", name="acc") for m in group}
                    for kp in range(KP):
                        for m in group:
                            nc.tensor.matmul(
                                gacc[m][:], slabs[m][:, 128 * kp:128 * (kp + 1)],
                                rtiles[kp][:], start=(kp == 0), stop=(kp == KP - 1))
                    for m in group:
                        ps = gacc[m]
                        if m < QH or m == QH:  # q heads and k need rope
                            sb = ep.tile([128, TBLK], BF16, tag="qk_sb", name="qk_sb")
                            nc.scalar.activation(sb[:], ps[:], AF.Copy)
                            ps_swap = ps_tmp.tile([128, TBLK], F32, tag="tmp", name="swp")
                            nc.tensor.matmul(ps_swap[:], swap_sb[:], sb[:],
                                             start=True, stop=True)
                            dst = qrot[m] if m < QH else krot
                            rope(dst, sb, ps_swap, sl)
                        else:  # v: plain copy
                            nc.scalar.activation(vsb[:, sl], ps[:], AF.Copy)
                # transpose this block's v chunks to token-major
                for q4 in range(BPT * NB // 128):
                    i = (TBLK * nb) // 128 + q4
                    psv = ps_tmp.tile([128, 512], BF16, tag="tmp", name="vtp")
                    nc.tensor.transpose(
                        psv[:, 128 * (i % 4):128 * (i % 4) + 128],
                        vsb[:, 128 * i:128 * (i + 1)], identb_sb[:])
                    nc.vector.tensor_copy(
                        vtok[:, 128 * i:128 * (i + 1)],
                        psv[:, 128 * (i % 4):128 * (i % 4) + 128])

        # ================= P2: attention =================
        with ExitStack() as ctx:
            pp = ctx.enter_context(tc.tile_pool(name="att_p", bufs=6))
            ap2 = ctx.enter_context(tc.tile_pool(name="att_t", bufs=4))
            for h in range(QH):
                for j in range(QT):
                    qsl = slice(TBLK * j, TBLK * (j + 1))
                    nk = (TBLK * (j + 1)) // DH
                    ps_a = ps_acc.tile([128, TBLK], F32, tag="acc", name="acc")
                    ps_l = ps_sml.tile([1, TBLK], F32, tag="lsum", name="lsum")
                    kpj = TBLK // DH  # k chunks per q tile (straddle count)
                    for i in range(nk):
                        ps_s = ps_tmp.tile([128, TBLK], F32, tag="tmp", name="score")
                        diagonal = i >= kpj * j
                        nc.tensor.matmul(
                            ps_s[:], krot[:, DH * i:DH * (i + 1)], qrot[h][:, qsl],
                            start=True, stop=not diagonal)
                        if diagonal:
                            ri = i - kpj * j
                            nc.tensor.matmul(
                                ps_s[:], diag_sb[:],
                                masks_sb[:, TBLK * ri:TBLK * (ri + 1)],
                                start=False, stop=True)
                        pt = pp.tile([128, TBLK], BF16, tag="p", name="p")
                        nc.scalar.activation(pt[:], ps_s[:], AF.Exp, scale=scale)
                        nc.tensor.matmul(ps_a[:], vtok[:, DH * i:DH * (i + 1)], pt[:],
                                         start=(i == 0), stop=(i == nk - 1))
                        nc.tensor.matmul(ps_l[:], ones_sb[:, 0:1], pt[:],
                                         start=(i == 0), stop=(i == nk - 1))
                    lrec_f = ap2.tile([1, TBLK], F32, tag="lrec_f", name="lrec_f")
                    nc.vector.reciprocal_approx_fast(lrec_f[:], ps_l[:])
                    lrec = ap2.tile([1, TBLK], BF16, tag="lrec", name="lrec")
                    with nc.allow_low_precision(reason="1/l broadcast via bf16 matmul"):
                        nc.vector.tensor_copy(lrec[:], lrec_f[:])
                    ps_b = ps_tmp.tile([128, TBLK], F32, tag="tmp", name="bcast")
                    nc.tensor.matmul(ps_b[:], ones_sb[0:1, :], lrec[:],
                                     start=True, stop=True)
                    linv = ap2.tile([128, TBLK], F32, tag="linv", name="linv")
                    nc.scalar.activation(linv[:], ps_b[:], AF.Copy)
                    nc.vector.tensor_tensor(aT[h][:, qsl], ps_a[:], linv[:],
                                            op=ALU.mult)

        # ================= P3: out-projection -> opart (chunked RS) ========
        opart_views = [opart_ch[ch][:].rearrange("(b d) t -> d b t", b=C)
                       for ch in range(nch)]
        with ExitStack() as ctx:
            wop = ctx.enter_context(tc.tile_pool(name="wo_w", bufs=6))
            oev = ctx.enter_context(tc.tile_pool(name="wo_ev", bufs=6))
            for m in range(KP):
                slab = wop.tile([128, QH * 128], BF16, tag="wo", name="wo")
                nc.sync.dma_start(
                    slab[:].rearrange("p (k m) -> p k m", m=128),
                    woT.ap()[:, 128 * m:128 * (m + 1)]
                    .rearrange("(k p) m -> p k m", p=128))
                ch, ml = ch_of(RS1_CUTS, m)
                for nb in range(NBLK):
                    ps = ps_acc.tile([128, TBLK], F32, tag="acc", name="acc")
                    for kp in range(QH):
                        nc.tensor.matmul(
                            ps[:], slab[:, 128 * kp:128 * (kp + 1)],
                            aT[kp][:, TBLK * nb:TBLK * (nb + 1)],
                            start=(kp == 0), stop=(kp == QH - 1))
                    ev = oev.tile([128, TBLK], RSDT, tag="ev", name="ev")
                    if (m + nb) % 2 == 0:
                        nc.scalar.activation(ev[:], ps[:], AF.Copy)
                    else:
                        nc.vector.tensor_copy(ev[:], ps[:])
                    nc.gpsimd.dma_start(
                        opart_views[ch][128 * ml:128 * (ml + 1),
                                        BPT * nb:BPT * (nb + 1), :],
                        ev[:].rearrange("p (b t) -> p b t", b=BPT))
                if m == RS1_CUTS[ch + 1] - 1:
                    nc.gpsimd.collective_compute(
                        "ReduceScatter", ALU.add,
                        replica_groups=[list(range(C))],
                        ins=[opart_ch[ch][:].opt()],
                        outs=[ored_ch[ch][:].opt()])
        attn_ctx.close()

        def transpose_add(src_chunks, cuts, pool, prefix, dst_tiles=None,
                          out_dram=None):
            """src_chunks (feature-major per-chunk) + residual -> token-major.
            If dst_tiles given: dst_tiles[t][:, gsl] = src.T + x_c  (P4)
            If out_dram given:  out_dram[t rows, gsl] = src.T + r2  (P6)"""
            tid = identb_sb if RS_BF16 else identf_sb
            for ch in range(len(cuts) - 1):
                for t in range(TT):
                    for gl in range((cuts[ch + 1] - cuts[ch]) * 128 // 512):
                        g = cuts[ch] * 128 // 512 + gl
                        gsl = slice(512 * g, 512 * (g + 1))
                        lt = pool.tile([128, 512], RSDT, tag=f"{prefix}lt", name=f"{prefix}lt")
                        nc.sync.dma_start(
                            lt[:].rearrange("p (q t) -> p q t", q=4),
                            src_chunks[ch][512 * gl:512 * (gl + 1),
                                           128 * t:128 * (t + 1)]
                            .rearrange("(q d) t -> d q t", q=4))
                        ps = ps_tmp.tile([128, 512], RSDT, tag="tmp", name="tps")
                        for q4 in range(4):
                            nc.tensor.transpose(
                                ps[:, 128 * q4:128 * (q4 + 1)],
                                lt[:, 128 * q4:128 * (q4 + 1)], tid[:])
                        if dst_tiles is not None:
                            xt_s = pool.tile([128, 512], F32, tag=f"{prefix}xs", name=f"{prefix}xs")
                            nc.sync.dma_start(
                                xt_s[:], x_c.ap()[128 * t:128 * (t + 1), gsl])
                            nc.vector.tensor_tensor(dst_tiles[t][:, gsl], ps[:],
                                                    xt_s[:], op=ALU.add)
                        else:
                            rsld = pool.tile([128, 512], F32, tag=f"{prefix}rs", name=f"{prefix}rs")
                            nc.sync.dma_start(
                                rsld[:], r2d[128 * t:128 * (t + 1), gsl])
                            ot = pool.tile([128, 512], F32, tag=f"{prefix}ot", name=f"{prefix}ot")
                            nc.vector.tensor_tensor(ot[:], ps[:],
                                                    rsld[:], op=ALU.add)
                            nc.gpsimd.dma_start(
                                out_dram[128 * t:128 * (t + 1), gsl], ot[:])

        # ================= P4: residual + norm2 + AllGather(h2) ============
        with ExitStack() as ctx:
            p4 = ctx.enter_context(tc.tile_pool(name="p4", bufs=2))
            resid = ctx.enter_context(tc.tile_pool(name="resid", bufs=1))
            r2_sb = [resid.tile([128, D], F32, tag=f"r2_{t}", name=f"r2_{t}")
                     for t in range(TT)]
            transpose_add(ored_ch, RS1_CUTS, p4, "p4", dst_tiles=r2_sb)
            seqpar_norm_and_gather(r2_sb, h2T_in_ch, h2T_all_ch, p4, ps_tmp, "n2")
            for t in range(TT):
                nc.sync.dma_start(r2d[128 * t:128 * (t + 1), :], r2_sb[t][:])

        # ================= P5: FFN =================
        h2_views = [h2T_all_ch[ch][:].rearrange("(b d) t -> d b t", b=C)
                    for ch in range(len(AG_CUTS) - 1)]
        fpart_views = [fpart_ch[ch][:].rearrange("(b d) t -> d b t", b=C)
                       for ch in range(nch)]
        with ExitStack() as ctx:
            frhs = ctx.enter_context(tc.tile_pool(name="ffn_rhs", bufs=1))
            fwp = ctx.enter_context(tc.tile_pool(name="ffn_w", bufs=2))
            fev = ctx.enter_context(tc.tile_pool(name="ffn_ev", bufs=3))
            ftp = ctx.enter_context(tc.tile_pool(name="ffn_fT", bufs=1))
            fTs = [ftp.tile([128, N], BF16, tag=f"fT{m}", name=f"fT{m}")
                   for m in range(FM)]
            for hf in range(2):
                rts = []
                for kp in range(KP):
                    rt = frhs.tile([128, T2], BF16, tag=f"rhs{kp}", name=f"rhs{kp}")
                    chq, kpl = ch_of(AG_CUTS, kp)
                    nc.sync.dma_start(
                        rt[:].rearrange("p (b t) -> p b t", b=BPH),
                        h2_views[chq][128 * kpl:128 * (kpl + 1),
                                      BPH * hf:BPH * (hf + 1), :])
                    rts.append(rt)
                for m in range(FM):
                    wg_s = fwp.tile([128, KP * 128], BF16, tag="wg", name="wg")
                    nc.sync.dma_start(
                        wg_s[:].rearrange("p (k m) -> p k m", m=128),
                        wgT.ap()[:, 128 * m:128 * (m + 1)]
                        .rearrange("(k p) m -> p k m", p=128))
                    wh_s = fwp.tile([128, KP * 128], BF16, tag="wh", name="wh")
                    nc.sync.dma_start(
                        wh_s[:].rearrange("p (k m) -> p k m", m=128),
                        whT.ap()[:, 128 * m:128 * (m + 1)]
                        .rearrange("(k p) m -> p k m", p=128))
                    for ns in range(NS2):
                        ssl = slice(TBLK * ns, TBLK * (ns + 1))
                        osl = slice(T2 * hf + TBLK * ns,
                                    T2 * hf + TBLK * (ns + 1))
                        ps_g = ps_acc.tile([128, TBLK], F32, tag="acc", name="acc")
                        ps_u = ps_acc.tile([128, TBLK], F32, tag="acc", name="acc")
                        for kp in range(KP):
                            nc.tensor.matmul(
                                ps_g[:], wg_s[:, 128 * kp:128 * (kp + 1)],
                                rts[kp][:, ssl], start=(kp == 0),
                                stop=(kp == KP - 1))
                            nc.tensor.matmul(
                                ps_u[:], wh_s[:, 128 * kp:128 * (kp + 1)],
                                rts[kp][:, ssl], start=(kp == 0),
                                stop=(kp == KP - 1))
                        gs = fev.tile([128, TBLK], F32, tag="gs", name="gs")
                        if SILU_VIA_SIGMOID:
                            nc.scalar.activation(gs[:], ps_g[:], AF.Sigmoid)
                            gg = fev.tile([128, TBLK], F32, tag="gg", name="gg")
                            nc.vector.tensor_tensor(gg[:], ps_g[:], gs[:],
                                                    op=ALU.mult)
                            gs = gg
                        else:
                            nc.scalar.activation(gs[:], ps_g[:], AF.Silu)
                        nc.vector.tensor_tensor(fTs[m][:, osl], gs[:], ps_u[:],
                                                op=ALU.mult)
            # combined wf pass over all tokens; RS chunks fire at feature
            # boundaries and overlap the rest of the pass
            for m2 in range(KP):
                wf_s = fwp.tile([128, FM * 128], BF16, tag="wf", name="wf")
                nc.sync.dma_start(
                    wf_s[:].rearrange("p (k m) -> p k m", m=128),
                    wfT.ap()[:, 128 * m2:128 * (m2 + 1)]
                    .rearrange("(k p) m -> p k m", p=128))
                ch2, m2l = ch_of(RS2_CUTS, m2)
                for ns in range(N // TBLK):
                    ssl = slice(TBLK * ns, TBLK * (ns + 1))
                    ps = ps_acc.tile([128, TBLK], F32, tag="acc", name="acc")
                    for kp in range(FM):
                        nc.tensor.matmul(
                            ps[:], wf_s[:, 128 * kp:128 * (kp + 1)],
                            fTs[kp][:, ssl], start=(kp == 0),
                            stop=(kp == FM - 1))
                    ev = fev.tile([128, TBLK], RSDT, tag="fv", name="fv")
                    if (m2 + ns) % 2 == 0:
                        nc.scalar.activation(ev[:], ps[:], AF.Copy)
                    else:
                        nc.vector.tensor_copy(ev[:], ps[:])
                    b0 = BPS * ns
                    nc.gpsimd.dma_start(
                        fpart_views[ch2][128 * m2l:128 * (m2l + 1),
                                         b0:b0 + BPS, :],
                        ev[:].rearrange("p (b t) -> p b t", b=BPS))
                if m2 == RS2_CUTS[ch2 + 1] - 1:
                    nc.gpsimd.collective_compute(
                        "ReduceScatter", ALU.add,
                        replica_groups=[list(range(C))],
                        ins=[fpart_ch[ch2][:].opt()],
                        outs=[fred_ch[ch2][:].opt()])

        # ================= P6: final residual add -> out =================
        with ExitStack() as ctx:
            p6 = ctx.enter_context(tc.tile_pool(name="p6", bufs=2))
            transpose_add(fred_ch, RS2_CUTS, p6, "p6", out_dram=out_c.ap())

    nc.compile()
    return nc


def make_in_maps(cfg, inputs):
    """Shard + transform the full fp32 inputs into per-core input maps."""
    N, D, QH, FC = cfg['N'], cfg['D'], cfg['QH'], cfg['FC']
    C = CORES
    NB = N // C
    bf = ml_dtypes.bfloat16
    f32 = np.float32

    x = np.ascontiguousarray(inputs['x'], dtype=f32)
    anw = np.asarray(inputs['attn_norm_w'], dtype=f32)
    fnw = np.asarray(inputs['ffn_norm_w'], dtype=f32)
    wq = np.asarray(inputs['wq'], dtype=f32) * anw[None, :]
    wk = np.asarray(inputs['wk'], dtype=f32) * anw[None, :]
    wv = np.asarray(inputs['wv'], dtype=f32) * anw[None, :]
    wo = np.asarray(inputs['wo'], dtype=f32)
    wg = np.asarray(inputs['wg'], dtype=f32) * fnw[None, :]
    wh = np.asarray(inputs['wh'], dtype=f32) * fnw[None, :]
    wf = np.asarray(inputs['wf'], dtype=f32)
    rcosT = np.ascontiguousarray(np.asarray(inputs['r_cos'], dtype=f32).T)
    rsinT = np.ascontiguousarray(np.asarray(inputs['r_sin'], dtype=f32).T)

    # rope swap as a matmul: swap(x) = P @ x ; lhsT = P.T
    P = np.zeros((DH, DH), dtype=f32)
    for i in range(DH // 2):
        P[2 * i, 2 * i + 1] = -1.0
        P[2 * i + 1, 2 * i] = 1.0
    swapT = np.ascontiguousarray(P.T)

    diagneg = np.diag(np.full(DH, NEG_BIG, dtype=f32))
    ident = np.eye(128, dtype=f32)
    ones = np.ones((128, 128), dtype=f32)
    m4 = np.zeros((4, 128, TBLK), dtype=f32)
    for ri in range(4):
        kk = np.arange(128)[:, None] + 128 * ri
        qq = np.arange(TBLK)[None, :]
        m4[ri] = (kk > qq).astype(f32)

    in_maps = []
    for c in range(C):
        qh_rows = slice(QH * DH * c, QH * DH * (c + 1))
        kv_rows = slice(DH * c, DH * (c + 1))
        fc_rows = slice(FC * c, FC * (c + 1))
        in_maps.append({
            "x_c": np.ascontiguousarray(x[NB * c:NB * (c + 1), :]),
            "wqT": np.ascontiguousarray(wq[qh_rows, :].T).astype(bf),
            "wkT": np.ascontiguousarray(wk[kv_rows, :].T).astype(bf),
            "wvT": np.ascontiguousarray(wv[kv_rows, :].T).astype(bf),
            "woT": np.ascontiguousarray(wo[:, qh_rows].T).astype(bf),
            "wgT": np.ascontiguousarray(wg[fc_rows, :].T).astype(bf),
            "whT": np.ascontiguousarray(wh[fc_rows, :].T).astype(bf),
            "wfT": np.ascontiguousarray(wf[:, fc_rows].T).astype(bf),
            "rcosT": rcosT,
            "rsinT": rsinT,
            "swapT": swapT.astype(bf),
            "diagneg": diagneg.astype(bf),
            "identb": ident.astype(bf),
            "identf": ident,
            "onesc": ones.astype(bf),
            "masks": m4.astype(bf),
        })
    return in_maps


def assemble(results):
    return np.concatenate([r["out_c"] for r in results], axis=0)


_NC_CACHE = {}


def get_module(cfg_key=None):
    cfg = FULL_CFG if cfg_key is None else cfg_key
    key = tuple(sorted(cfg.items()))
    if key not in _NC_CACHE:
        _NC_CACHE[key] = build_module(cfg)
    return _NC_CACHE[key]


def run(inputs, cfg=None, trace=False):
    cfg = cfg or FULL_CFG
    nc = get_module(cfg)
    in_maps = make_in_maps(cfg, inputs)
    r = run_bass_kernel_spmd(nc, in_maps, list(range(CORES)), trace=trace)
    return assemble(r.results), r


def kernel(**inputs):
    out, _ = run(inputs)
    return np.asarray(out, dtype=np.float32)



# revision 4
# speedup vs baseline: 1.1334x; 1.1334x over previous
"""Tensor-parallel Llama layer on 8 Trainium2 NeuronCores (Bass/Tile).

Sharding: TP per the hint — wq/wk/wv/wg/wh column-sharded (4 q-heads + 1 kv
head + 1792 ffn rows per core), wo/wf row-sharded with ReduceScatter after
attention-out and ffn-out; sequence-parallel RMSNorms (256 tokens/core) with
AllGather of the normed activations (bf16).

Activations are kept feature-major (x.T layout) on chip so every projection
is a plain lhsT.T @ rhs with contraction on the partition axis. Weights are
pre-transposed and pre-cast to bf16 on the host (host prep is free).
"""
import sys

sys.path.insert(0, '/opt/trn_rl_repo')
from contextlib import ExitStack

import numpy as np
import ml_dtypes

import concourse.bass as bass
import concourse.tile as tile
from concourse import bacc, mybir
from concourse.bass_utils import run_bass_kernel_spmd

AF = mybir.ActivationFunctionType
ALU = mybir.AluOpType
BF16 = mybir.dt.bfloat16
F32 = mybir.dt.float32

CORES = 8
DH = 128
EPS = 1e-5
TBLK = 512
NEG_BIG = -1e30

FULL_CFG = dict(N=2048, D=4096, QH=4, FC=1792)

# CoreSim doesn't implement Silu; set True to build with Sigmoid + an extra
# multiply (same math) for simulator validation.
SILU_VIA_SIGMOID = False

# ReduceScatter the attention/ffn partial sums in bf16 (halves collective
# time); flip to False if accuracy needs the headroom.
RS_BF16 = True

# Feature-chunks per collective (pipelines collectives behind compute).
NCH = 4


def build_module(cfg):
    N, D, QH, FC = cfg['N'], cfg['D'], cfg['QH'], cfg['FC']
    C = CORES
    NB = N // C            # tokens per core block
    TT = NB // 128         # token tiles per core block
    KP = D // 128          # d_model contraction chunks
    NBLK = N // TBLK       # matmul token blocks
    BPT = TBLK // NB       # DRAM token-blocks per matmul token block
    KCH = N // DH          # attention k chunks
    QT = N // TBLK         # q tiles per head
    FM = FC // DH          # ffn M tiles per core
    T2 = N // 2            # ffn token half
    NS2 = T2 // TBLK       # 512-subblocks per ffn half
    BPH = C // 2           # DRAM token-blocks per ffn half
    BPS = TBLK // NB       # DRAM token-blocks per 512-subblock
    MQKV = QH + 2
    scale = float(1.0 / np.sqrt(DH))

    nc = bacc.Bacc("TRN2", target_bir_lowering=False, debug=False, num_devices=C)

    x_c = nc.dram_tensor("x_c", [NB, D], F32, kind="ExternalInput")
    wqT = nc.dram_tensor("wqT", [D, QH * DH], BF16, kind="ExternalInput")
    wkT = nc.dram_tensor("wkT", [D, DH], BF16, kind="ExternalInput")
    wvT = nc.dram_tensor("wvT", [D, DH], BF16, kind="ExternalInput")
    woT = nc.dram_tensor("woT", [QH * DH, D], BF16, kind="ExternalInput")
    wgT = nc.dram_tensor("wgT", [D, FC], BF16, kind="ExternalInput")
    whT = nc.dram_tensor("whT", [D, FC], BF16, kind="ExternalInput")
    wfT = nc.dram_tensor("wfT", [FC, D], BF16, kind="ExternalInput")
    rcosT = nc.dram_tensor("rcosT", [DH, N], F32, kind="ExternalInput")
    rsinT = nc.dram_tensor("rsinT", [DH, N], F32, kind="ExternalInput")
    swapT = nc.dram_tensor("swapT", [DH, DH], BF16, kind="ExternalInput")
    diagneg = nc.dram_tensor("diagneg", [DH, DH], BF16, kind="ExternalInput")
    identb = nc.dram_tensor("identb", [128, 128], BF16, kind="ExternalInput")
    identf = nc.dram_tensor("identf", [128, 128], F32, kind="ExternalInput")
    onesc = nc.dram_tensor("onesc", [128, 128], BF16, kind="ExternalInput")
    masks = nc.dram_tensor("masks", [4, 128, TBLK], BF16, kind="ExternalInput")
    out_c = nc.dram_tensor("out_c", [NB, D], F32, kind="ExternalOutput")

    RSDT = BF16 if RS_BF16 else F32
    nch = max(1, min(NCH, D // 512))   # effective chunk count
    DCH = D // nch          # features per collective chunk
    KPC = KP // nch         # kp (128-feature) tiles per chunk
    assert KP % nch == 0 and DCH % 512 == 0
    even = [KPC * i for i in range(nch + 1)]
    # AllGathers: uneven chunks — tiny first chunk so the consumer (QKV/FFN)
    # starts as early as possible; the bulk follows while compute runs.
    AG_CUTS = [0, 4, 16, KP] if KP >= 32 else [0, KP]
    RS1_CUTS = even
    # ReduceScatter for ffn-out: small LAST chunk so the end-of-kernel
    # exposed tail (last RS + residual add) is short.
    RS2_CUTS = [0, 12, 20, 28, KP] if KP >= 32 else even

    def ch_of(cuts, kp):
        for c in range(len(cuts) - 1):
            if kp < cuts[c + 1]:
                return c, kp - cuts[c]
        raise ValueError

    with tile.TileContext(nc) as tc, ExitStack() as top:
        dram = top.enter_context(tc.tile_pool(name="dram", bufs=1, space="DRAM"))

        def dram_chunks(nm, cuts, mul, dt, shared=False):
            kw = dict(addr_space="Shared") if shared else {}
            return [dram.tile([(cuts[i + 1] - cuts[i]) * 128 * mul, NB], dt,
                              tag=f"{nm}{i}", name=f"{nm}{i}", **kw)
                    for i in range(len(cuts) - 1)]

        r2d = dram.tile([NB, D], F32, tag="r2d", name="r2d")
        hT_in_ch = dram_chunks("hT_in", AG_CUTS, 1, BF16)
        hT_all_ch = dram_chunks("hT_all", AG_CUTS, C, BF16, shared=True)
        opart_ch = dram_chunks("opart", RS1_CUTS, C, RSDT)
        ored_ch = dram_chunks("ored", RS1_CUTS, 1, RSDT)
        h2T_in_ch = dram_chunks("h2T_in", AG_CUTS, 1, BF16)
        h2T_all_ch = dram_chunks("h2T_all", AG_CUTS, C, BF16, shared=True)
        fpart_ch = dram_chunks("fpart", RS2_CUTS, C, RSDT)
        fred_ch = dram_chunks("fred", RS2_CUTS, 1, RSDT)

        # ---- constants resident in SBUF ----
        const = top.enter_context(tc.tile_pool(name="const", bufs=1))
        swap_sb = const.tile([DH, DH], BF16, tag="swap", name="swap")
        diag_sb = const.tile([DH, DH], BF16, tag="diag", name="diag")
        identb_sb = const.tile([128, 128], BF16, tag="identb", name="identb")
        identf_sb = const.tile([128, 128], F32, tag="identf", name="identf")
        ones_sb = const.tile([128, 128], BF16, tag="ones", name="ones")
        masks_sb = const.tile([128, 4 * TBLK], BF16, tag="masks", name="masks")
        nc.sync.dma_start(swap_sb[:], swapT.ap())
        nc.sync.dma_start(diag_sb[:], diagneg.ap())
        nc.sync.dma_start(identb_sb[:], identb.ap())
        nc.sync.dma_start(identf_sb[:], identf.ap())
        nc.sync.dma_start(ones_sb[:], onesc.ap())
        nc.sync.dma_start(
            masks_sb[:].rearrange("p (r t) -> p r t", r=4),
            masks.ap().rearrange("r p t -> p r t"),
        )

        # ---- shared PSUM pools (total 4+3+1 = 8 banks) ----
        ps_acc = top.enter_context(tc.tile_pool(name="ps_acc", bufs=4, space="PSUM"))
        ps_tmp = top.enter_context(tc.tile_pool(name="ps_tmp", bufs=3, space="PSUM"))
        ps_sml = top.enter_context(tc.tile_pool(name="ps_sml", bufs=1, space="PSUM"))

        # ---- attention residents (freed after P3; opened last for LIFO) ----
        attn_ctx = ExitStack()
        attn = attn_ctx.enter_context(tc.tile_pool(name="attn", bufs=1))
        rcos_sb = attn.tile([DH, N], F32, tag="rcos", name="rcos")
        rsin_sb = attn.tile([DH, N], F32, tag="rsin", name="rsin")
        nc.sync.dma_start(rcos_sb[:], rcosT.ap())
        nc.sync.dma_start(rsin_sb[:], rsinT.ap())
        qrot = [attn.tile([DH, N], BF16, tag=f"qrot{h}", name=f"qrot{h}") for h in range(QH)]
        krot = attn.tile([DH, N], BF16, tag="krot", name="krot")
        vsb = attn.tile([DH, N], BF16, tag="vsb", name="vsb")
        vtok = attn.tile([128, KCH * DH], BF16, tag="vtok", name="vtok")
        aT = [attn.tile([DH, N], BF16, tag=f"aT{h}", name=f"aT{h}") for h in range(QH)]

        def seqpar_norm_and_gather(src_tiles, dst_chunks, out_chunks, pool,
                                   pspool, prefix):
            """src_tiles: TT SBUF tiles [128, D] f32 (token-major rows of this
            core's block). RMS-normalize each row, transpose to feature-major
            chunk tensors [DCH, NB], then AllGather each chunk."""
            for t in range(TT):
                xt = src_tiles[t]
                sq = pool.tile([128, D], F32, tag=f"{prefix}sq", name=f"{prefix}sq")
                ssum = pool.tile([128, 1], F32, tag=f"{prefix}ss", name=f"{prefix}ss")
                nc.scalar.activation(sq[:], xt[:], AF.Square, accum_out=ssum[:])
                var = pool.tile([128, 1], F32, tag=f"{prefix}var", name=f"{prefix}var")
                nc.vector.tensor_scalar(
                    out=var[:], in0=ssum[:], scalar1=1.0 / D, scalar2=EPS,
                    op0=ALU.mult, op1=ALU.add)
                sv = pool.tile([128, 1], F32, tag=f"{prefix}sv", name=f"{prefix}sv")
                nc.scalar.activation(sv[:], var[:], AF.Sqrt)
                rstd = pool.tile([128, 1], F32, tag=f"{prefix}rstd", name=f"{prefix}rstd")
                nc.vector.reciprocal(rstd[:], sv[:])
                htok = pool.tile([128, D], BF16, tag=f"{prefix}h", name=f"{prefix}h")
                nc.vector.tensor_scalar_mul(htok[:], xt[:], rstd[:])
                for g in range(D // 512):
                    ps = pspool.tile([128, 512], BF16, tag="tmp", name="tps")
                    for q4 in range(4):
                        dd = 4 * g + q4
                        nc.tensor.transpose(
                            ps[:, 128 * q4:128 * (q4 + 1)],
                            htok[:, 128 * dd:128 * (dd + 1)], identb_sb[:])
                    ev = pool.tile([128, 512], BF16, tag=f"{prefix}ev", name=f"{prefix}ev")
                    if g % 2 == 0:
                        nc.vector.tensor_copy(ev[:], ps[:])
                    else:
                        nc.scalar.activation(ev[:], ps[:], AF.Copy)
                    ch, kpl0 = ch_of(AG_CUTS, 4 * g)
                    r0 = 128 * kpl0
                    nc.gpsimd.dma_start(
                        dst_chunks[ch][r0:r0 + 512, 128 * t:128 * (t + 1)]
                        .rearrange("(q d) t -> d q t", q=4),
                        ev[:].rearrange("p (q t) -> p q t", q=4))
            for ch in range(len(AG_CUTS) - 1):
                nc.gpsimd.collective_compute(
                    "AllGather", ALU.bypass, replica_groups=[list(range(C))],
                    ins=[dst_chunks[ch][:].opt()], outs=[out_chunks[ch][:].opt()])

        # ================= P0: norm1 (seq-parallel) + chunked AllGather ====
        with ExitStack() as ctx:
            p0 = ctx.enter_context(tc.tile_pool(name="p0", bufs=1))
            x_tiles = []
            for t in range(TT):
                xt = p0.tile([128, D], F32, tag=f"x{t}", name=f"x{t}")
                nc.sync.dma_start(xt[:], x_c.ap()[128 * t:128 * (t + 1), :])
                x_tiles.append(xt)
            seqpar_norm_and_gather(x_tiles, hT_in_ch, hT_all_ch, p0, ps_tmp, "n1")

        hT_views = [hT_all_ch[ch][:].rearrange("(b d) t -> d b t", b=C)
                    for ch in range(len(AG_CUTS) - 1)]

        # ================= P1: QKV + RoPE (per token block) =================
        with ExitStack() as ctx:
            wsl = ctx.enter_context(tc.tile_pool(name="qkv_w", bufs=1))
            rhsp = ctx.enter_context(tc.tile_pool(name="qkv_rhs", bufs=1))
            ep = ctx.enter_context(tc.tile_pool(name="qkv_ep", bufs=3))
            # QKV weight slabs are small (6 x 8KB/partition bf16): load once
            slabs = []
            for m in range(MQKV):
                slab = wsl.tile([128, KP * 128], BF16, tag=f"w{m}", name=f"w{m}")
                if m < QH:
                    src = wqT.ap()[:, 128 * m:128 * (m + 1)]
                elif m == QH:
                    src = wkT.ap()
                else:
                    src = wvT.ap()
                nc.sync.dma_start(
                    slab[:].rearrange("p (k m) -> p k m", m=128),
                    src.rearrange("(k p) m -> p k m", p=128))
                slabs.append(slab)

            def rope(dst, src_sb, ps_swap, sl):
                """dst[:, sl] = src*cos + (P@src)*sin; src_sb bf16, ps_swap psum."""
                t1 = ep.tile([128, TBLK], F32, tag="rope_t1", name="rope_t1")
                nc.vector.tensor_tensor(t1[:], src_sb[:], rcos_sb[:, sl], op=ALU.mult)
                t2 = ep.tile([128, TBLK], F32, tag="rope_t2", name="rope_t2")
                nc.vector.tensor_tensor(t2[:], ps_swap[:], rsin_sb[:, sl], op=ALU.mult)
                nc.vector.tensor_tensor(dst[:, sl], t1[:], t2[:], op=ALU.add)

            for nb in range(NBLK):
                sl = slice(TBLK * nb, TBLK * (nb + 1))
                # one rhs load per (nb, kp), shared by both M-groups
                rtiles = []
                for kp in range(KP):
                    rt = rhsp.tile([128, TBLK], BF16, tag=f"rhs{kp}", name=f"rhs{kp}")
                    chq, kpl = ch_of(AG_CUTS, kp)
                    nc.sync.dma_start(
                        rt[:].rearrange("p (b t) -> p b t", b=BPT),
                        hT_views[chq][128 * kpl:128 * (kpl + 1),
                                      BPT * nb:BPT * (nb + 1), :])
                    rtiles.append(rt)
                for hm in range(2):
                    group = list(range(3 * hm, min(3 * (hm + 1), MQKV)))
                    gacc = {m: ps_acc.tile([128, TBLK], F32, tag="acc", name="acc") for m in group}
                    for kp in range(KP):
                        for m in group:
                            nc.tensor.matmul(
                                gacc[m][:], slabs[m][:, 128 * kp:128 * (kp + 1)],
                                rtiles[kp][:], start=(kp == 0), stop=(kp == KP - 1))
                    for m in group:
                        ps = gacc[m]
                        if m < QH or m == QH:  # q heads and k need rope
                            sb = ep.tile([128, TBLK], BF16, tag="qk_sb", name="qk_sb")
                            nc.scalar.activation(sb[:], ps[:], AF.Copy)
                            ps_swap = ps_tmp.tile([128, TBLK], F32, tag="tmp", name="swp")
                            nc.tensor.matmul(ps_swap[:], swap_sb[:], sb[:],
                                             start=True, stop=True)
                            dst = qrot[m] if m < QH else krot
                            rope(dst, sb, ps_swap, sl)
                        else:  # v: plain copy
                            nc.scalar.activation(vsb[:, sl], ps[:], AF.Copy)
                # transpose this block's v chunks to token-major
                for q4 in range(BPT * NB // 128):
                    i = (TBLK * nb) // 128 + q4
                    psv = ps_tmp.tile([128, 512], BF16, tag="tmp", name="vtp")
                    nc.tensor.transpose(
                        psv[:, 128 * (i % 4):128 * (i % 4) + 128],
                        vsb[:, 128 * i:128 * (i + 1)], identb_sb[:])
                    nc.vector.tensor_copy(
                        vtok[:, 128 * i:128 * (i + 1)],
                        psv[:, 128 * (i % 4):128 * (i % 4) + 128])

        # ================= P2: attention =================
        with ExitStack() as ctx:
            pp = ctx.enter_context(tc.tile_pool(name="att_p", bufs=6))
            ap2 = ctx.enter_context(tc.tile_pool(name="att_t", bufs=4))
            for h in range(QH):
                for j in range(QT):
                    qsl = slice(TBLK * j, TBLK * (j + 1))
                    nk = (TBLK * (j + 1)) // DH
                    ps_a = ps_acc.tile([128, TBLK], F32, tag="acc", name="acc")
                    ps_l = ps_sml.tile([1, TBLK], F32, tag="lsum", name="lsum")
                    kpj = TBLK // DH  # k chunks per q tile (straddle count)
                    for i in range(nk):
                        ps_s = ps_tmp.tile([128, TBLK], F32, tag="tmp", name="score")
                        diagonal = i >= kpj * j
                        nc.tensor.matmul(
                            ps_s[:], krot[:, DH * i:DH * (i + 1)], qrot[h][:, qsl],
                            start=True, stop=not diagonal)
                        if diagonal:
                            ri = i - kpj * j
                            nc.tensor.matmul(
                                ps_s[:], diag_sb[:],
                                masks_sb[:, TBLK * ri:TBLK * (ri + 1)],
                                start=False, stop=True)
                        pt = pp.tile([128, TBLK], BF16, tag="p", name="p")
                        nc.scalar.activation(pt[:], ps_s[:], AF.Exp, scale=scale)
                        nc.tensor.matmul(ps_a[:], vtok[:, DH * i:DH * (i + 1)], pt[:],
                                         start=(i == 0), stop=(i == nk - 1))
                        nc.tensor.matmul(ps_l[:], ones_sb[:, 0:1], pt[:],
                                         start=(i == 0), stop=(i == nk - 1))
                    lrec_f = ap2.tile([1, TBLK], F32, tag="lrec_f", name="lrec_f")
                    nc.vector.reciprocal_approx_fast(lrec_f[:], ps_l[:])
                    lrec = ap2.tile([1, TBLK], BF16, tag="lrec", name="lrec")
                    with nc.allow_low_precision(reason="1/l broadcast via bf16 matmul"):
                        nc.vector.tensor_copy(lrec[:], lrec_f[:])
                    ps_b = ps_acc.tile([128, TBLK], F32, tag="acc", name="bcast")
                    nc.tensor.matmul(ps_b[:], ones_sb[0:1, :], lrec[:],
                                     start=True, stop=True)
                    linv = ap2.tile([128, TBLK], F32, tag="linv", name="linv")
                    nc.scalar.activation(linv[:], ps_b[:], AF.Copy)
                    nc.vector.tensor_tensor(aT[h][:, qsl], ps_a[:], linv[:],
                                            op=ALU.mult)

        # ================= P3: out-projection -> opart (chunked RS) ========
        opart_views = [opart_ch[ch][:].rearrange("(b d) t -> d b t", b=C)
                       for ch in range(nch)]
        with ExitStack() as ctx:
            wop = ctx.enter_context(tc.tile_pool(name="wo_w", bufs=6))
            oev = ctx.enter_context(tc.tile_pool(name="wo_ev", bufs=6))
            for m in range(KP):
                slab = wop.tile([128, QH * 128], BF16, tag="wo", name="wo")
                nc.sync.dma_start(
                    slab[:].rearrange("p (k m) -> p k m", m=128),
                    woT.ap()[:, 128 * m:128 * (m + 1)]
                    .rearrange("(k p) m -> p k m", p=128))
                ch, ml = ch_of(RS1_CUTS, m)
                for nb in range(NBLK):
                    ps = ps_acc.tile([128, TBLK], F32, tag="acc", name="acc")
                    for kp in range(QH):
                        nc.tensor.matmul(
                            ps[:], slab[:, 128 * kp:128 * (kp + 1)],
                            aT[kp][:, TBLK * nb:TBLK * (nb + 1)],
                            start=(kp == 0), stop=(kp == QH - 1))
                    ev = oev.tile([128, TBLK], RSDT, tag="ev", name="ev")
                    if (m + nb) % 2 == 0:
                        nc.scalar.activation(ev[:], ps[:], AF.Copy)
                    else:
                        nc.vector.tensor_copy(ev[:], ps[:])
                    nc.gpsimd.dma_start(
                        opart_views[ch][128 * ml:128 * (ml + 1),
                                        BPT * nb:BPT * (nb + 1), :],
                        ev[:].rearrange("p (b t) -> p b t", b=BPT))
                if m == RS1_CUTS[ch + 1] - 1:
                    nc.gpsimd.collective_compute(
                        "ReduceScatter", ALU.add,
                        replica_groups=[list(range(C))],
                        ins=[opart_ch[ch][:].opt()],
                        outs=[ored_ch[ch][:].opt()])
        attn_ctx.close()

        def transpose_add(src_chunks, cuts, pool, prefix, dst_tiles=None,
                          out_dram=None):
            """src_chunks (feature-major per-chunk) + residual -> token-major.
            If dst_tiles given: dst_tiles[t][:, gsl] = src.T + x_c  (P4)
            If out_dram given:  out_dram[t rows, gsl] = src.T + r2  (P6)"""
            tid = identb_sb if RS_BF16 else identf_sb
            for ch in range(len(cuts) - 1):
                for t in range(TT):
                    for gl in range((cuts[ch + 1] - cuts[ch]) * 128 // 512):
                        g = cuts[ch] * 128 // 512 + gl
                        gsl = slice(512 * g, 512 * (g + 1))
                        lt = pool.tile([128, 512], RSDT, tag=f"{prefix}lt", name=f"{prefix}lt")
                        nc.sync.dma_start(
                            lt[:].rearrange("p (q t) -> p q t", q=4),
                            src_chunks[ch][512 * gl:512 * (gl + 1),
                                           128 * t:128 * (t + 1)]
                            .rearrange("(q d) t -> d q t", q=4))
                        ps = ps_tmp.tile([128, 512], RSDT, tag="tmp", name="tps")
                        for q4 in range(4):
                            nc.tensor.transpose(
                                ps[:, 128 * q4:128 * (q4 + 1)],
                                lt[:, 128 * q4:128 * (q4 + 1)], tid[:])
                        if dst_tiles is not None:
                            xt_s = pool.tile([128, 512], F32, tag=f"{prefix}xs", name=f"{prefix}xs")
                            nc.sync.dma_start(
                                xt_s[:], x_c.ap()[128 * t:128 * (t + 1), gsl])
                            nc.vector.tensor_tensor(dst_tiles[t][:, gsl], ps[:],
                                                    xt_s[:], op=ALU.add)
                        else:
                            rsld = pool.tile([128, 512], F32, tag=f"{prefix}rs", name=f"{prefix}rs")
                            nc.sync.dma_start(
                                rsld[:], r2d[128 * t:128 * (t + 1), gsl])
                            ot = pool.tile([128, 512], F32, tag=f"{prefix}ot", name=f"{prefix}ot")
                            nc.vector.tensor_tensor(ot[:], ps[:],
                                                    rsld[:], op=ALU.add)
                            nc.gpsimd.dma_start(
                                out_dram[128 * t:128 * (t + 1), gsl], ot[:])

        # ================= P4: residual + norm2 + AllGather(h2) ============
        with ExitStack() as ctx:
            p4 = ctx.enter_context(tc.tile_pool(name="p4", bufs=2))
            resid = ctx.enter_context(tc.tile_pool(name="resid", bufs=1))
            r2_sb = [resid.tile([128, D], F32, tag=f"r2_{t}", name=f"r2_{t}")
                     for t in range(TT)]
            transpose_add(ored_ch, RS1_CUTS, p4, "p4", dst_tiles=r2_sb)
            seqpar_norm_and_gather(r2_sb, h2T_in_ch, h2T_all_ch, p4, ps_tmp, "n2")
            for t in range(TT):
                nc.sync.dma_start(r2d[128 * t:128 * (t + 1), :], r2_sb[t][:])

        # ================= P5: FFN =================
        h2_views = [h2T_all_ch[ch][:].rearrange("(b d) t -> d b t", b=C)
                    for ch in range(len(AG_CUTS) - 1)]
        fpart_views = [fpart_ch[ch][:].rearrange("(b d) t -> d b t", b=C)
                       for ch in range(nch)]
        with ExitStack() as ctx:
            frhs = ctx.enter_context(tc.tile_pool(name="ffn_rhs", bufs=1))
            fwp = ctx.enter_context(tc.tile_pool(name="ffn_w", bufs=2))
            fev = ctx.enter_context(tc.tile_pool(name="ffn_ev", bufs=3))
            ftp = ctx.enter_context(tc.tile_pool(name="ffn_fT", bufs=1))
            fTs = [ftp.tile([128, N], BF16, tag=f"fT{m}", name=f"fT{m}")
                   for m in range(FM)]
            for hf in range(2):
                rts = []
                for kp in range(KP):
                    rt = frhs.tile([128, T2], BF16, tag=f"rhs{kp}", name=f"rhs{kp}")
                    chq, kpl = ch_of(AG_CUTS, kp)
                    nc.sync.dma_start(
                        rt[:].rearrange("p (b t) -> p b t", b=BPH),
                        h2_views[chq][128 * kpl:128 * (kpl + 1),
                                      BPH * hf:BPH * (hf + 1), :])
                    rts.append(rt)
                for m in range(FM):
                    wg_s = fwp.tile([128, KP * 128], BF16, tag="wg", name="wg")
                    nc.sync.dma_start(
                        wg_s[:].rearrange("p (k m) -> p k m", m=128),
                        wgT.ap()[:, 128 * m:128 * (m + 1)]
                        .rearrange("(k p) m -> p k m", p=128))
                    wh_s = fwp.tile([128, KP * 128], BF16, tag="wh", name="wh")
                    nc.sync.dma_start(
                        wh_s[:].rearrange("p (k m) -> p k m", m=128),
                        whT.ap()[:, 128 * m:128 * (m + 1)]
                        .rearrange("(k p) m -> p k m", p=128))
                    for ns in range(NS2):
                        ssl = slice(TBLK * ns, TBLK * (ns + 1))
                        osl = slice(T2 * hf + TBLK * ns,
                                    T2 * hf + TBLK * (ns + 1))
                        ps_g = ps_acc.tile([128, TBLK], F32, tag="acc", name="acc")
                        ps_u = ps_acc.tile([128, TBLK], F32, tag="acc", name="acc")
                        for kp in range(KP):
                            nc.tensor.matmul(
                                ps_g[:], wg_s[:, 128 * kp:128 * (kp + 1)],
                                rts[kp][:, ssl], start=(kp == 0),
                                stop=(kp == KP - 1))
                            nc.tensor.matmul(
                                ps_u[:], wh_s[:, 128 * kp:128 * (kp + 1)],
                                rts[kp][:, ssl], start=(kp == 0),
                                stop=(kp == KP - 1))
                        gs = fev.tile([128, TBLK], F32, tag="gs", name="gs")
                        if SILU_VIA_SIGMOID:
                            nc.scalar.activation(gs[:], ps_g[:], AF.Sigmoid)
                            gg = fev.tile([128, TBLK], F32, tag="gg", name="gg")
                            nc.vector.tensor_tensor(gg[:], ps_g[:], gs[:],
                                                    op=ALU.mult)
                            gs = gg
                        else:
                            nc.scalar.activation(gs[:], ps_g[:], AF.Silu)
                        nc.vector.tensor_tensor(fTs[m][:, osl], gs[:], ps_u[:],
                                                op=ALU.mult)
            # combined wf pass over all tokens; RS chunks fire at feature
            # boundaries and overlap the rest of the pass
            for m2 in range(KP):
                wf_s = fwp.tile([128, FM * 128], BF16, tag="wf", name="wf")
                nc.sync.dma_start(
                    wf_s[:].rearrange("p (k m) -> p k m", m=128),
                    wfT.ap()[:, 128 * m2:128 * (m2 + 1)]
                    .rearrange("(k p) m -> p k m", p=128))
                ch2, m2l = ch_of(RS2_CUTS, m2)
                for ns in range(N // TBLK):
                    ssl = slice(TBLK * ns, TBLK * (ns + 1))
                    ps = ps_acc.tile([128, TBLK], F32, tag="acc", name="acc")
                    for kp in range(FM):
                        nc.tensor.matmul(
                            ps[:], wf_s[:, 128 * kp:128 * (kp + 1)],
                            fTs[kp][:, ssl], start=(kp == 0),
                            stop=(kp == FM - 1))
                    ev = fev.tile([128, TBLK], RSDT, tag="fv", name="fv")
                    if (m2 + ns) % 2 == 0:
                        nc.scalar.activation(ev[:], ps[:], AF.Copy)
                    else:
                        nc.vector.tensor_copy(ev[:], ps[:])
                    b0 = BPS * ns
                    nc.gpsimd.dma_start(
                        fpart_views[ch2][128 * m2l:128 * (m2l + 1),
                                         b0:b0 + BPS, :],
                        ev[:].rearrange("p (b t) -> p b t", b=BPS))
                if m2 == RS2_CUTS[ch2 + 1] - 1:
                    nc.gpsimd.collective_compute(
                        "ReduceScatter", ALU.add,
                        replica_groups=[list(range(C))],
                        ins=[fpart_ch[ch2][:].opt()],
                        outs=[fred_ch[ch2][:].opt()])

        # ================= P6: final residual add -> out =================
        with ExitStack() as ctx:
            p6 = ctx.enter_context(tc.tile_pool(name="p6", bufs=2))
            transpose_add(fred_ch, RS2_CUTS, p6, "p6", out_dram=out_c.ap())

    nc.compile()
    return nc


def make_in_maps(cfg, inputs):
    """Shard + transform the full fp32 inputs into per-core input maps."""
    N, D, QH, FC = cfg['N'], cfg['D'], cfg['QH'], cfg['FC']
    C = CORES
    NB = N // C
    bf = ml_dtypes.bfloat16
    f32 = np.float32

    x = np.ascontiguousarray(inputs['x'], dtype=f32)
    anw = np.asarray(inputs['attn_norm_w'], dtype=f32)
    fnw = np.asarray(inputs['ffn_norm_w'], dtype=f32)
    wq = np.asarray(inputs['wq'], dtype=f32) * anw[None, :]
    wk = np.asarray(inputs['wk'], dtype=f32) * anw[None, :]
    wv = np.asarray(inputs['wv'], dtype=f32) * anw[None, :]
    wo = np.asarray(inputs['wo'], dtype=f32)
    wg = np.asarray(inputs['wg'], dtype=f32) * fnw[None, :]
    wh = np.asarray(inputs['wh'], dtype=f32) * fnw[None, :]
    wf = np.asarray(inputs['wf'], dtype=f32)
    rcosT = np.ascontiguousarray(np.asarray(inputs['r_cos'], dtype=f32).T)
    rsinT = np.ascontiguousarray(np.asarray(inputs['r_sin'], dtype=f32).T)

    # rope swap as a matmul: swap(x) = P @ x ; lhsT = P.T
    P = np.zeros((DH, DH), dtype=f32)
    for i in range(DH // 2):
        P[2 * i, 2 * i + 1] = -1.0
        P[2 * i + 1, 2 * i] = 1.0
    swapT = np.ascontiguousarray(P.T)

    diagneg = np.diag(np.full(DH, NEG_BIG, dtype=f32))
    ident = np.eye(128, dtype=f32)
    ones = np.ones((128, 128), dtype=f32)
    m4 = np.zeros((4, 128, TBLK), dtype=f32)
    for ri in range(4):
        kk = np.arange(128)[:, None] + 128 * ri
        qq = np.arange(TBLK)[None, :]
        m4[ri] = (kk > qq).astype(f32)

    in_maps = []
    for c in range(C):
        qh_rows = slice(QH * DH * c, QH * DH * (c + 1))
        kv_rows = slice(DH * c, DH * (c + 1))
        fc_rows = slice(FC * c, FC * (c + 1))
        in_maps.append({
            "x_c": np.ascontiguousarray(x[NB * c:NB * (c + 1), :]),
            "wqT": np.ascontiguousarray(wq[qh_rows, :].T).astype(bf),
            "wkT": np.ascontiguousarray(wk[kv_rows, :].T).astype(bf),
            "wvT": np.ascontiguousarray(wv[kv_rows, :].T).astype(bf),
            "woT": np.ascontiguousarray(wo[:, qh_rows].T).astype(bf),
            "wgT": np.ascontiguousarray(wg[fc_rows, :].T).astype(bf),
            "whT": np.ascontiguousarray(wh[fc_rows, :].T).astype(bf),
            "wfT": np.ascontiguousarray(wf[:, fc_rows].T).astype(bf),
            "rcosT": rcosT,
            "rsinT": rsinT,
            "swapT": swapT.astype(bf),
            "diagneg": diagneg.astype(bf),
            "identb": ident.astype(bf),
            "identf": ident,
            "onesc": ones.astype(bf),
            "masks": m4.astype(bf),
        })
    return in_maps


def assemble(results):
    return np.concatenate([r["out_c"] for r in results], axis=0)


_NC_CACHE = {}


def get_module(cfg_key=None):
    cfg = FULL_CFG if cfg_key is None else cfg_key
    key = tuple(sorted(cfg.items()))
    if key not in _NC_CACHE:
        _NC_CACHE[key] = build_module(cfg)
    return _NC_CACHE[key]


def run(inputs, cfg=None, trace=False):
    cfg = cfg or FULL_CFG
    nc = get_module(cfg)
    in_maps = make_in_maps(cfg, inputs)
    r = run_bass_kernel_spmd(nc, in_maps, list(range(CORES)), trace=trace)
    return assemble(r.results), r


def kernel(**inputs):
    out, _ = run(inputs)
    return np.asarray(out, dtype=np.float32)



# revision 5
# speedup vs baseline: 1.1349x; 1.0013x over previous
"""Tensor-parallel Llama layer on 8 Trainium2 NeuronCores (Bass/Tile), v4.

Key structure (per core c):
- seq-parallel norm1 in feature-major layout; AllGather of normed x in 4
  uneven feature chunks (tiny first chunk so QKV starts early).
- QKV (4 q-heads + 1 kv head) + RoPE; attention over all 2048 tokens.
- per-head AllGather of attention outputs (overlaps attention); the
  out-projection is FEATURE-sharded: each core computes its 512-feature
  slice of wo @ a for all tokens (no ReduceScatter chain).
- norm2 feature-parallel: per-token sum-of-squares partials AllReduced
  (2KB per 512-token block), overlapped with out-proj compute.
- FFN as in v1 (column-sharded wg/wh, row-sharded wf) but the final
  ReduceScatter scatters along FEATURES, so the output stays
  feature-major end-to-end (no transposes); last chunk split into
  token-quarters for a short tail.

Perf-critical mechanics learned from traces:
- every dma_start costs ~1.1us on the issuing queue -> batch DMAs into
  single multi-dim-AP transfers wherever possible.
- collectives serialize on the gpsimd queue; nothing else goes there.
- PE LDWEIGHTS is hidden by hardware; no stationary amortization needed.
"""
import sys

sys.path.insert(0, '/opt/trn_rl_repo')
from contextlib import ExitStack

import numpy as np
import ml_dtypes

import concourse.bass as bass
import concourse.tile as tile
from concourse import bacc, mybir
from concourse.bass_utils import run_bass_kernel_spmd

AF = mybir.ActivationFunctionType
ALU = mybir.AluOpType
BF16 = mybir.dt.bfloat16
F32 = mybir.dt.float32

CORES = 8
DH = 128
EPS = 1e-5
TBLK = 512
NEG_BIG = -1e30

FULL_CFG = dict(N=2048, D=4096, QH=4, FC=1792)


def build_module(cfg):
    N, D, QH, FC = cfg['N'], cfg['D'], cfg['QH'], cfg['FC']
    C = CORES
    NB = N // C            # tokens per core block
    KP = D // 128          # d_model contraction chunks
    NBLK = N // TBLK       # matmul token blocks
    BPT = TBLK // NB       # DRAM token-blocks per matmul token block
    KCH = N // DH          # attention k chunks
    QT = N // TBLK         # q tiles per head
    FM = FC // DH          # ffn M tiles per core
    T2 = N // 2            # ffn token half
    NS2 = T2 // TBLK       # 512-subblocks per ffn half
    FS = D // C            # output-feature slice per core
    FT = FS // 128         # 128-feature tiles per slice
    MQKV = QH + 2
    scale = float(1.0 / np.sqrt(DH))

    nc = bacc.Bacc("TRN2", target_bir_lowering=False, debug=False, num_devices=C)

    xT_own = nc.dram_tensor("xT_own", [D, NB], F32, kind="ExternalInput")
    xT_fs = nc.dram_tensor("xT_fs", [FS, N], F32, kind="ExternalInput")
    wqT = nc.dram_tensor("wqT", [D, QH * DH], BF16, kind="ExternalInput")
    wkT = nc.dram_tensor("wkT", [D, DH], BF16, kind="ExternalInput")
    wvT = nc.dram_tensor("wvT", [D, DH], BF16, kind="ExternalInput")
    woTc = nc.dram_tensor("woTc", [D, FS], BF16, kind="ExternalInput")
    wgT = nc.dram_tensor("wgT", [D, FC], BF16, kind="ExternalInput")
    whT = nc.dram_tensor("whT", [D, FC], BF16, kind="ExternalInput")
    wfT = nc.dram_tensor("wfT", [FC, D], BF16, kind="ExternalInput")
    rcosT = nc.dram_tensor("rcosT", [DH, N], BF16, kind="ExternalInput")
    rsinT = nc.dram_tensor("rsinT", [DH, N], BF16, kind="ExternalInput")
    swapT = nc.dram_tensor("swapT", [DH, DH], BF16, kind="ExternalInput")
    diagneg = nc.dram_tensor("diagneg", [DH, DH], BF16, kind="ExternalInput")
    identb = nc.dram_tensor("identb", [128, 128], BF16, kind="ExternalInput")
    onesc = nc.dram_tensor("onesc", [128, 128], BF16, kind="ExternalInput")
    onesf = nc.dram_tensor("onesf", [1, 128], F32, kind="ExternalInput")
    masks = nc.dram_tensor("masks", [4, 128, TBLK], BF16, kind="ExternalInput")
    out_c = nc.dram_tensor("out_c", [FS, N], F32, kind="ExternalOutput")

    # hT AllGather chunks (kp-tile units): QKV can't really run ahead of the
    # gather anyway, so minimize total AG wall time: one small starter chunk
    # plus one big chunk
    AG_CUTS = [0, 4, KP]
    NAG = len(AG_CUTS) - 1

    def ch_of(cuts, kp):
        for c in range(len(cuts) - 1):
            if kp < cuts[c + 1]:
                return c, kp - cuts[c]
        raise ValueError

    with tile.TileContext(nc) as tc, ExitStack() as top:
        dram = top.enter_context(tc.tile_pool(name="dram", bufs=1, space="DRAM"))

        hT_in_ch = [dram.tile([(AG_CUTS[i + 1] - AG_CUTS[i]) * 128, NB], BF16,
                              tag=f"hTi{i}", name=f"hTi{i}")
                    for i in range(NAG)]
        hT_all_ch = [dram.tile([(AG_CUTS[i + 1] - AG_CUTS[i]) * 128 * C, NB], BF16,
                               tag=f"hTa{i}", name=f"hTa{i}", addr_space="Shared")
                     for i in range(NAG)]
        aT_in = [dram.tile([DH, N], BF16, tag=f"aTi{h}", name=f"aTi{h}")
                 for h in range(QH)]
        aT_all = [dram.tile([C * DH, N], BF16, tag=f"aTa{h}", name=f"aTa{h}",
                            addr_space="Shared") for h in range(QH)]
        # last head's aT gathered in two token-half chunks (earlier P3 start)
        aT3_in = [dram.tile([DH, T2], BF16, tag=f"aT3i{v}", name=f"aT3i{v}")
                  for v in range(2)]
        aT3_all = [dram.tile([C * DH, T2], BF16, tag=f"aT3a{v}", name=f"aT3a{v}",
                             addr_space="Shared") for v in range(2)]
        ar_in = [dram.tile([1, 2 * TBLK], F32, tag=f"ari{t}", name=f"ari{t}")
                 for t in range(2)]
        ar_out = [dram.tile([1, 2 * TBLK], F32, tag=f"aro{t}", name=f"aro{t}",
                            addr_space="Shared") for t in range(2)]
        h2_in = [dram.tile([FS, T2], BF16, tag=f"h2i{h}", name=f"h2i{h}")
                 for h in range(2)]
        h2_all = [dram.tile([C * FS, T2], BF16, tag=f"h2a{h}", name=f"h2a{h}",
                            addr_space="Shared") for h in range(2)]
        # ffn-out RS chunks: one per (feature-offset, token half); the very
        # last (off=FT-1, half=1) further split into two token quarters
        fp2, fr2 = {}, {}
        for o in range(FT):
            for v in range(2):
                if o == FT - 1 and v == 1:
                    continue
                fp2[(o, v)] = dram.tile([C * 128, T2], BF16,
                                        tag=f"fp{o}_{v}", name=f"fp{o}_{v}")
                fr2[(o, v)] = dram.tile([128, T2], BF16,
                                        tag=f"fr{o}_{v}", name=f"fr{o}_{v}")
        fpq = [dram.tile([C * 128, TBLK], BF16, tag=f"fpq{v}", name=f"fpq{v}")
               for v in range(2)]
        frq = [dram.tile([128, TBLK], BF16, tag=f"frq{v}", name=f"frq{v}")
               for v in range(2)]
        r2d = dram.tile([FS, N], F32, tag="r2d", name="r2d")

        # ---- constants resident in SBUF ----
        const = top.enter_context(tc.tile_pool(name="const", bufs=1))
        swap_sb = const.tile([DH, DH], BF16, tag="swap", name="swap")
        diag_sb = const.tile([DH, DH], BF16, tag="diag", name="diag")
        identb_sb = const.tile([128, 128], BF16, tag="identb", name="identb")
        ones_sb = const.tile([128, 128], BF16, tag="ones", name="ones")
        onesf_sb = const.tile([1, 128], F32, tag="onesf", name="onesf")
        masks_sb = const.tile([128, 4 * TBLK], BF16, tag="masks", name="masks")
        nc.sync.dma_start(swap_sb[:], swapT.ap())
        nc.sync.dma_start(diag_sb[:], diagneg.ap())
        nc.sync.dma_start(identb_sb[:], identb.ap())
        nc.sync.dma_start(ones_sb[:], onesc.ap())
        nc.sync.dma_start(onesf_sb[:], onesf.ap())
        nc.sync.dma_start(
            masks_sb[:].rearrange("p (r t) -> p r t", r=4),
            masks.ap().rearrange("r p t -> p r t"),
        )

        # ---- shared PSUM pools (4+3+1 = 8 banks) ----
        ps_acc = top.enter_context(tc.tile_pool(name="ps_acc", bufs=4, space="PSUM"))
        ps_tmp = top.enter_context(tc.tile_pool(name="ps_tmp", bufs=3, space="PSUM"))
        ps_sml = top.enter_context(tc.tile_pool(name="ps_sml", bufs=1, space="PSUM"))

        # ---- P3 resident: wo slab (DMA issued during P2, freed after P3) --
        p3res_ctx = ExitStack()
        p3res = p3res_ctx.enter_context(tc.tile_pool(name="p3res", bufs=1))
        woT_sb = p3res.tile([128, KP * FS], BF16, tag="woc", name="woc")

        # ---- attention residents (freed after P2) ----
        attn_ctx = ExitStack()
        attn = attn_ctx.enter_context(tc.tile_pool(name="attn", bufs=1))
        atw = attn_ctx.enter_context(tc.tile_pool(name="atw", bufs=2))
        rcos_sb = attn.tile([DH, N], BF16, tag="rcos", name="rcos")
        rsin_sb = attn.tile([DH, N], BF16, tag="rsin", name="rsin")
        nc.sync.dma_start(rcos_sb[:], rcosT.ap())
        nc.sync.dma_start(rsin_sb[:], rsinT.ap())
        qrot = [attn.tile([DH, N], BF16, tag=f"qrot{h}", name=f"qrot{h}") for h in range(QH)]
        krot = attn.tile([DH, N], BF16, tag="krot", name="krot")
        vsb = attn.tile([DH, N], BF16, tag="vsb", name="vsb")
        vtok = attn.tile([128, KCH * DH], BF16, tag="vtok", name="vtok")

        # ================= P0: norm1 (feature-major, seq-parallel) + AG ====
        with ExitStack() as ctx:
            p0 = ctx.enter_context(tc.tile_pool(name="p0", bufs=1))
            xbig = p0.tile([128, KP * NB], F32, tag="xbig", name="xbig")
            nc.scalar.dma_start(
                xbig[:].rearrange("p (k t) -> p k t", k=KP),
                xT_own.ap().rearrange("(k p) t -> p k t", p=128))
            sqb = p0.tile([128, KP * NB], BF16, tag="sqb", name="sqb")
            ps_ss = ps_sml.tile([1, NB], F32, tag="lsum", name="ss0")
            for kp in range(KP):
                ksl = slice(NB * kp, NB * (kp + 1))
                nc.scalar.activation(sqb[:, ksl], xbig[:, ksl], AF.Square)
                nc.tensor.matmul(ps_ss[:], ones_sb[:, 0:1], sqb[:, ksl],
                                 start=(kp == 0), stop=(kp == KP - 1))
            var = p0.tile([1, NB], F32, tag="var", name="var")
            nc.vector.tensor_scalar(out=var[:], in0=ps_ss[:], scalar1=1.0 / D,
                                    scalar2=EPS, op0=ALU.mult, op1=ALU.add)
            sd = p0.tile([1, NB], F32, tag="sd", name="sd")
            nc.scalar.activation(sd[:], var[:], AF.Sqrt)
            rstd = p0.tile([1, NB], F32, tag="rstd", name="rstd")
            nc.vector.reciprocal(rstd[:], sd[:])
            ps_bc = ps_tmp.tile([128, NB], F32, tag="tmp", name="bc0")
            nc.tensor.matmul(ps_bc[:], onesf_sb[:], rstd[:], start=True, stop=True)
            rstd_bc = p0.tile([128, NB], F32, tag="rbc", name="rbc")
            nc.scalar.activation(rstd_bc[:], ps_bc[:], AF.Copy)
            hbig = p0.tile([128, KP * NB], BF16, tag="hbig", name="hbig")
            for ch in range(NAG):
                for kp in range(AG_CUTS[ch], AG_CUTS[ch + 1]):
                    ksl = slice(NB * kp, NB * (kp + 1))
                    nc.vector.tensor_tensor(hbig[:, ksl], xbig[:, ksl],
                                            rstd_bc[:], op=ALU.mult)
                csl = slice(NB * AG_CUTS[ch], NB * AG_CUTS[ch + 1])
                nc.scalar.dma_start(
                    hT_in_ch[ch][:].rearrange("(k p) t -> p k t", p=128),
                    hbig[:, csl].rearrange("p (k t) -> p k t", t=NB))
                nc.gpsimd.collective_compute(
                    "AllGather", ALU.bypass, replica_groups=[list(range(C))],
                    ins=[hT_in_ch[ch][:].opt()], outs=[hT_all_ch[ch][:].opt()])

        # ================= P1: QKV + RoPE (per token block) =================
        with ExitStack() as ctx:
            wsl = ctx.enter_context(tc.tile_pool(name="qkv_w", bufs=1))
            rhsp = ctx.enter_context(tc.tile_pool(name="qkv_rhs", bufs=1))
            ep = ctx.enter_context(tc.tile_pool(name="qkv_ep", bufs=2))
            slabs = []
            for m in range(MQKV):
                slab = wsl.tile([128, KP * 128], BF16, tag=f"w{m}", name=f"w{m}")
                if m < QH:
                    src = wqT.ap()[:, 128 * m:128 * (m + 1)]
                elif m == QH:
                    src = wkT.ap()
                else:
                    src = wvT.ap()
                nc.sync.dma_start(
                    slab[:].rearrange("p (k m) -> p k m", m=128),
                    src.rearrange("(k p) m -> p k m", p=128))
                slabs.append(slab)

            def rope(dst, src_sb, ps_swap, sl):
                t1 = ep.tile([128, TBLK], F32, tag="rope_t1", name="rope_t1")
                nc.vector.tensor_tensor(t1[:], src_sb[:], rcos_sb[:, sl], op=ALU.mult)
                t2 = ep.tile([128, TBLK], F32, tag="rope_t2", name="rope_t2")
                nc.vector.tensor_tensor(t2[:], ps_swap[:], rsin_sb[:, sl], op=ALU.mult)
                nc.vector.tensor_tensor(dst[:, sl], t1[:], t2[:], op=ALU.add)

            def load_rtb_chunk(rtb, nb, ch):
                rtb_v = rtb[:].rearrange("p (k b t) -> p k b t", k=KP, b=BPT)
                src_v = hT_all_ch[ch][:].rearrange(
                    "(b k p) t -> p k b t", b=C, p=128)
                for b in range(BPT):
                    nc.sync.dma_start(
                        rtb_v[:, AG_CUTS[ch]:AG_CUTS[ch + 1], b, :],
                        src_v[:, :, BPT * nb + b, :])

            for nb in range(NBLK):
                sl = slice(TBLK * nb, TBLK * (nb + 1))
                rtb = rhsp.tile([128, KP * TBLK], BF16, tag=f"rtb{nb % 2}",
                                name=f"rtb{nb}")
                for ch in range(NAG):
                    load_rtb_chunk(rtb, nb, ch)
                for hm in range(2):
                    group = list(range(3 * hm, min(3 * (hm + 1), MQKV)))
                    gacc = {m: ps_acc.tile([128, TBLK], F32, tag="acc", name="acc") for m in group}
                    for kp in range(KP):
                        for m in group:
                            nc.tensor.matmul(
                                gacc[m][:], slabs[m][:, 128 * kp:128 * (kp + 1)],
                                rtb[:, TBLK * kp:TBLK * (kp + 1)],
                                start=(kp == 0), stop=(kp == KP - 1))
                    for m in group:
                        ps = gacc[m]
                        if m <= QH:  # q heads and k need rope
                            sb = ep.tile([128, TBLK], BF16, tag="qk_sb", name="qk_sb")
                            nc.scalar.activation(sb[:], ps[:], AF.Copy)
                            ps_swap = ps_tmp.tile([128, TBLK], F32, tag="tmp", name="swp")
                            nc.tensor.matmul(ps_swap[:], swap_sb[:], sb[:],
                                             start=True, stop=True)
                            dst = qrot[m] if m < QH else krot
                            rope(dst, sb, ps_swap, sl)
                        else:  # v: plain copy
                            nc.scalar.activation(vsb[:, sl], ps[:], AF.Copy)
                for q4 in range(BPT * NB // 128):
                    i = (TBLK * nb) // 128 + q4
                    psv = ps_tmp.tile([128, 512], BF16, tag="tmp", name="vtp")
                    nc.tensor.transpose(
                        psv[:, 128 * (i % 4):128 * (i % 4) + 128],
                        vsb[:, 128 * i:128 * (i + 1)], identb_sb[:])
                    nc.vector.tensor_copy(
                        vtok[:, 128 * i:128 * (i + 1)],
                        psv[:, 128 * (i % 4):128 * (i % 4) + 128])

        # ================= P2: attention (+ per-head AllGather of aT) ======
        with ExitStack() as ctx:
            pp = ctx.enter_context(tc.tile_pool(name="att_p", bufs=6))
            ap2 = ctx.enter_context(tc.tile_pool(name="att_t", bufs=4))
            # wo slab load rides the idle sync queue during attention
            nc.sync.dma_start(
                woT_sb[:].rearrange("p (k m) -> p k m", m=FS),
                woTc.ap().rearrange("(k p) m -> p k m", p=128))
            for h in range(QH):
                aTh = atw.tile([DH, N], BF16, tag="aTh", name=f"aTh{h}")
                for j in range(QT):
                    qsl = slice(TBLK * j, TBLK * (j + 1))
                    nk = (TBLK * (j + 1)) // DH
                    ps_a = ps_acc.tile([128, TBLK], F32, tag="acc", name="acc")
                    ps_l = ps_sml.tile([1, TBLK], F32, tag="lsum", name="lsum")
                    kpj = TBLK // DH
                    for i in range(nk):
                        ps_s = ps_tmp.tile([128, TBLK], F32, tag="tmp", name="score")
                        diagonal = i >= kpj * j
                        nc.tensor.matmul(
                            ps_s[:], krot[:, DH * i:DH * (i + 1)], qrot[h][:, qsl],
                            start=True, stop=not diagonal)
                        if diagonal:
                            ri = i - kpj * j
                            nc.tensor.matmul(
                                ps_s[:], diag_sb[:],
                                masks_sb[:, TBLK * ri:TBLK * (ri + 1)],
                                start=False, stop=True)
                        pt = pp.tile([128, TBLK], BF16, tag="p", name="p")
                        nc.scalar.activation(pt[:], ps_s[:], AF.Exp, scale=scale)
                        nc.tensor.matmul(ps_a[:], vtok[:, DH * i:DH * (i + 1)], pt[:],
                                         start=(i == 0), stop=(i == nk - 1))
                        nc.tensor.matmul(ps_l[:], ones_sb[:, 0:1], pt[:],
                                         start=(i == 0), stop=(i == nk - 1))
                    lrec_f = ap2.tile([1, TBLK], F32, tag="lrec_f", name="lrec_f")
                    nc.vector.reciprocal_approx_fast(lrec_f[:], ps_l[:])
                    lrec = ap2.tile([1, TBLK], BF16, tag="lrec", name="lrec")
                    with nc.allow_low_precision(reason="1/l broadcast via bf16 matmul"):
                        nc.vector.tensor_copy(lrec[:], lrec_f[:])
                    ps_b = ps_acc.tile([128, TBLK], F32, tag="acc", name="bcast")
                    nc.tensor.matmul(ps_b[:], ones_sb[0:1, :], lrec[:],
                                     start=True, stop=True)
                    linv = ap2.tile([128, TBLK], F32, tag="linv", name="linv")
                    nc.scalar.activation(linv[:], ps_b[:], AF.Copy)
                    nc.vector.tensor_tensor(aTh[:, qsl], ps_a[:], linv[:],
                                            op=ALU.mult)
                    # last head: gather each token half as soon as it's done
                    if h == QH - 1 and j % 2 == 1:
                        v = j // 2
                        csl = slice(T2 * v, T2 * (v + 1))
                        nc.sync.dma_start(aT3_in[v][:], aTh[:, csl])
                        nc.gpsimd.collective_compute(
                            "AllGather", ALU.bypass,
                            replica_groups=[list(range(C))],
                            ins=[aT3_in[v][:].opt()],
                            outs=[aT3_all[v][:].opt()])
                if h < QH - 1:
                    # ship this head's output and AllGather it (overlaps next)
                    nc.sync.dma_start(aT_in[h][:], aTh[:])
                    nc.gpsimd.collective_compute(
                        "AllGather", ALU.bypass, replica_groups=[list(range(C))],
                        ins=[aT_in[h][:].opt()], outs=[aT_all[h][:].opt()])
        attn_ctx.close()

        # ========== P3: feature-sharded out-proj + residual + norm2 ========
        # psum[m] accumulates head-major (h outer, source-core inner) so only
        # the last head's chunks wait on the final aT AllGather.  Per token
        # block: add x slice, square-sum, AllReduce partials; scale/AG for
        # blocks {0,1} issue after block 2's compute (PE never waits on AR).
        r2_sb = []   # [tb] -> [128, FT*TBLK] f32 (kept for h2 scaling)
        with ExitStack() as ctx:
            atp = ctx.enter_context(tc.tile_pool(name="p3_at", bufs=2))
            p3 = ctx.enter_context(tc.tile_pool(name="p3", bufs=1))
            p3t = ctx.enter_context(tc.tile_pool(name="p3t", bufs=2))
            xfs_sb = [p3.tile([128, N], F32, tag=f"xfs{i}", name=f"xfs{i}")
                      for i in range(FT)]
            for i in range(FT):
                nc.scalar.dma_start(xfs_sb[i][:],
                                    xT_fs.ap()[128 * i:128 * (i + 1), :])

            def p3_scale_pair(tbs):
                """rstd2 + h2 scaling for two token blocks, then AG the half."""
                for tb in tbs:
                    asum = p3t.tile([1, TBLK], F32, tag="asum", name="asum")
                    nc.sync.dma_start(
                        asum[:],
                        ar_out[tb // 2][:, TBLK * (tb % 2):TBLK * (tb % 2 + 1)])
                    var2 = p3t.tile([1, TBLK], F32, tag="var2", name="var2")
                    nc.vector.tensor_scalar(
                        out=var2[:], in0=asum[:], scalar1=1.0 / D,
                        scalar2=EPS, op0=ALU.mult, op1=ALU.add)
                    sd2 = p3t.tile([1, TBLK], F32, tag="sd2", name="sd2")
                    nc.scalar.activation(sd2[:], var2[:], AF.Sqrt)
                    rstd2 = p3t.tile([1, TBLK], F32, tag="rstd2", name="rstd2")
                    nc.vector.reciprocal(rstd2[:], sd2[:])
                    ps_bc = ps_tmp.tile([128, TBLK], F32, tag="tmp", name="bc2")
                    nc.tensor.matmul(ps_bc[:], onesf_sb[:], rstd2[:],
                                     start=True, stop=True)
                    rbc = p3t.tile([128, TBLK], F32, tag="rbc2", name="rbc2")
                    nc.scalar.activation(rbc[:], ps_bc[:], AF.Copy)
                    hf, col = tb // 2, (tb % 2) * TBLK
                    h2row = p3t.tile([128, FT * TBLK], BF16, tag="h2row",
                                     name="h2row")
                    for m in range(FT):
                        nc.vector.tensor_tensor(
                            h2row[:, TBLK * m:TBLK * (m + 1)],
                            r2_sb[tb][:, TBLK * m:TBLK * (m + 1)], rbc[:],
                            op=ALU.mult)
                    nc.scalar.dma_start(
                        h2_in[hf][:, col:col + TBLK]
                        .rearrange("(m p) t -> p m t", p=128),
                        h2row[:].rearrange("p (m t) -> p m t", m=FT))
                hf = tbs[0] // 2
                nc.gpsimd.collective_compute(
                    "AllGather", ALU.bypass, replica_groups=[list(range(C))],
                    ins=[h2_in[hf][:].opt()], outs=[h2_all[hf][:].opt()])

            for tb in range(NBLK):
                tsl = slice(TBLK * tb, TBLK * (tb + 1))
                ats = {}
                for h in range(QH):
                    ath = atp.tile([128, C * TBLK], BF16, tag=f"ath{h}",
                                   name=f"ath{h}")
                    if h == QH - 1:
                        src = aT3_all[tb // 2][:] \
                            .rearrange("(c p) t -> p c t", p=128) \
                            [:, :, slice(TBLK * (tb % 2), TBLK * (tb % 2 + 1))]
                    else:
                        src = aT_all[h][:].rearrange(
                            "(c p) t -> p c t", p=128)[:, :, tsl]
                    nc.sync.dma_start(
                        ath[:].rearrange("p (c t) -> p c t", c=C), src)
                    ats[h] = ath
                psm = [ps_acc.tile([128, TBLK], F32, tag="acc", name="acc")
                       for m in range(FT)]
                n_hc = 0
                for h in range(QH):
                    for c_src in range(C):
                        g = QH * c_src + h   # global head = woT slab block
                        for m in range(FT):
                            nc.tensor.matmul(
                                psm[m][:],
                                woT_sb[:, FS * g + 128 * m:FS * g + 128 * (m + 1)],
                                ats[h][:, TBLK * c_src:TBLK * (c_src + 1)],
                                start=(n_hc == 0), stop=(n_hc == C * QH - 1))
                        n_hc += 1
                if tb == 2:
                    # AR for blocks {0,1} has landed by now: scale + fire the
                    # first-half h2 AllGather before tb2/tb3 evictions
                    p3_scale_pair([0, 1])
                ps_ss = ps_sml.tile([1, TBLK], F32, tag="lsum", name=f"ss{tb}")
                r2row = p3.tile([128, FT * TBLK], F32, tag=f"r2_{tb}", name=f"r2_{tb}")
                sqrow = p3t.tile([128, FT * TBLK], BF16, tag="sqrow", name="sqrow")
                for m in range(FT):
                    msl = slice(TBLK * m, TBLK * (m + 1))
                    nc.vector.tensor_tensor(r2row[:, msl], psm[m][:],
                                            xfs_sb[m][:, tsl], op=ALU.add)
                    nc.scalar.activation(sqrow[:, msl], r2row[:, msl], AF.Square)
                    nc.tensor.matmul(ps_ss[:], ones_sb[:, 0:1], sqrow[:, msl],
                                     start=(m == 0), stop=(m == FT - 1))
                nc.sync.dma_start(
                    r2d[:, tsl].rearrange("(m p) t -> p m t", p=128),
                    r2row[:].rearrange("p (m t) -> p m t", m=FT))
                r2_sb.append(r2row)
                ssum = p3t.tile([1, TBLK], F32, tag="ssum", name="ssum")
                nc.vector.tensor_copy(ssum[:], ps_ss[:])
                nc.scalar.dma_start(
                    ar_in[tb // 2][:, TBLK * (tb % 2):TBLK * (tb % 2 + 1)],
                    ssum[:])
                if tb % 2 == 1:   # one AllReduce per token-block pair
                    nc.gpsimd.collective_compute(
                        "AllReduce", ALU.add, replica_groups=[list(range(C))],
                        ins=[ar_in[tb // 2][:].opt()],
                        outs=[ar_out[tb // 2][:].opt()])
            p3_scale_pair([2, 3])
        p3res_ctx.close()

        # ================= P5: FFN =================
        with ExitStack() as ctx:
            frhs = ctx.enter_context(tc.tile_pool(name="ffn_rhs", bufs=1))
            fwp = ctx.enter_context(tc.tile_pool(name="ffn_w", bufs=2))
            fwf = ctx.enter_context(tc.tile_pool(name="ffn_wf", bufs=5))
            fev = ctx.enter_context(tc.tile_pool(name="ffn_ev", bufs=3))
            ftp = ctx.enter_context(tc.tile_pool(name="ffn_fT", bufs=1))
            fTs = [ftp.tile([128, N], BF16, tag=f"fT{m}", name=f"fT{m}")
                   for m in range(FM)]

            def load_gu_slabs(m):
                wg_s = fwp.tile([128, KP * 128], BF16, tag="wg", name="wg")
                nc.sync.dma_start(
                    wg_s[:].rearrange("p (k m) -> p k m", m=128),
                    wgT.ap()[:, 128 * m:128 * (m + 1)]
                    .rearrange("(k p) m -> p k m", p=128))
                wh_s = fwp.tile([128, KP * 128], BF16, tag="wh", name="wh")
                nc.sync.dma_start(
                    wh_s[:].rearrange("p (k m) -> p k m", m=128),
                    whT.ap()[:, 128 * m:128 * (m + 1)]
                    .rearrange("(k p) m -> p k m", p=128))
                return wg_s, wh_s

            def gu_pass(hf):
                # first m-tile's weights before the (AG-gated) rhs; rhs in 8
                # batched per-core-slice DMAs split across sync+scalar queues
                slab0 = load_gu_slabs(0) if hf == 0 else None
                rt2 = frhs.tile([128, KP * T2], BF16, tag="rt2", name="rt2")
                for c8 in range(C):
                    eng = nc.sync if c8 % 2 == 0 else nc.scalar
                    eng.dma_start(
                        rt2[:, T2 * FT * c8:T2 * FT * (c8 + 1)]
                        .rearrange("p (m t) -> p m t", m=FT),
                        h2_all[hf][FS * c8:FS * (c8 + 1), :]
                        .rearrange("(m p) t -> p m t", p=128))
                for m in range(FM):
                    wg_s, wh_s = slab0 if (hf == 0 and m == 0) \
                        else load_gu_slabs(m)
                    for ns in range(NS2):
                        osl = slice(T2 * hf + TBLK * ns,
                                    T2 * hf + TBLK * (ns + 1))
                        ps_g = ps_acc.tile([128, TBLK], F32, tag="acc", name="acc")
                        ps_u = ps_acc.tile([128, TBLK], F32, tag="acc", name="acc")
                        for kp in range(KP):
                            rsl = slice(T2 * kp + TBLK * ns,
                                        T2 * kp + TBLK * (ns + 1))
                            nc.tensor.matmul(
                                ps_g[:], wg_s[:, 128 * kp:128 * (kp + 1)],
                                rt2[:, rsl], start=(kp == 0),
                                stop=(kp == KP - 1))
                            nc.tensor.matmul(
                                ps_u[:], wh_s[:, 128 * kp:128 * (kp + 1)],
                                rt2[:, rsl], start=(kp == 0),
                                stop=(kp == KP - 1))
                        gs = fev.tile([128, TBLK], BF16, tag="gs", name="gs")
                        nc.scalar.activation(gs[:], ps_g[:], AF.Silu)
                        nc.vector.tensor_tensor(fTs[m][:, osl], gs[:], ps_u[:],
                                                op=ALU.mult)

            def wf_pass(hv):
                # offset-major over this token half; RS per (off, half) so
                # half-0 collectives drain under half-1's gate/up compute
                base_ns = [2 * hv, 2 * hv + 1]
                for off in range(FT):
                    last = off == FT - 1 and hv == 1
                    subchunks = [[base_ns[0]], [base_ns[1]]] if last \
                        else [base_ns]
                    for si, nsr in enumerate(subchunks):
                        for c8 in range(C):
                            m2 = FT * c8 + off
                            wf_s = fwf.tile([128, FM * 128], BF16, tag="wf",
                                            name="wf")
                            nc.sync.dma_start(
                                wf_s[:].rearrange("p (k m) -> p k m", m=128),
                                wfT.ap()[:, 128 * m2:128 * (m2 + 1)]
                                .rearrange("(k p) m -> p k m", p=128))
                            fprow = fev.tile([128, len(nsr) * TBLK], BF16,
                                             tag=f"fprow{len(nsr)}", name="fprow")
                            for n_i, ns in enumerate(nsr):
                                ssl = slice(TBLK * ns, TBLK * (ns + 1))
                                esl = slice(TBLK * n_i, TBLK * (n_i + 1))
                                ps = ps_acc.tile([128, TBLK], F32, tag="acc",
                                                 name="acc")
                                for kp in range(FM):
                                    nc.tensor.matmul(
                                        ps[:], wf_s[:, 128 * kp:128 * (kp + 1)],
                                        fTs[kp][:, ssl], start=(kp == 0),
                                        stop=(kp == FM - 1))
                                if (m2 + ns) % 2 == 0:
                                    nc.scalar.activation(fprow[:, esl], ps[:],
                                                         AF.Copy)
                                else:
                                    nc.vector.tensor_copy(fprow[:, esl], ps[:])
                            if last:
                                dst = fpq[si][128 * c8:128 * (c8 + 1), :]
                            else:
                                dst = fp2[(off, hv)][128 * c8:128 * (c8 + 1), :]
                            nc.scalar.dma_start(dst, fprow[:])
                        if last:
                            nc.gpsimd.collective_compute(
                                "ReduceScatter", ALU.add,
                                replica_groups=[list(range(C))],
                                ins=[fpq[si][:].opt()], outs=[frq[si][:].opt()])
                        else:
                            nc.gpsimd.collective_compute(
                                "ReduceScatter", ALU.add,
                                replica_groups=[list(range(C))],
                                ins=[fp2[(off, hv)][:].opt()],
                                outs=[fr2[(off, hv)][:].opt()])

            gu_pass(0)
            wf_pass(0)
            gu_pass(1)
            wf_pass(1)

        # ================= P6: final residual add -> out =================
        with ExitStack() as ctx:
            p6 = ctx.enter_context(tc.tile_pool(name="p6", bufs=2))
            for off in range(FT):
                frow = p6.tile([128, N], BF16, tag="p6f", name="p6f")
                nc.sync.dma_start(frow[:, :T2], fr2[(off, 0)][:])
                if off == FT - 1:
                    for si in range(2):
                        nc.sync.dma_start(
                            frow[:, T2 + TBLK * si:T2 + TBLK * (si + 1)],
                            frq[si][:])
                else:
                    nc.sync.dma_start(frow[:, T2:], fr2[(off, 1)][:])
                rrow = p6.tile([128, N], F32, tag="p6r", name="p6r")
                nc.sync.dma_start(rrow[:], r2d[128 * off:128 * (off + 1), :])
                orow = p6.tile([128, N], F32, tag="p6o", name="p6o")
                nc.vector.tensor_tensor(orow[:], frow[:], rrow[:], op=ALU.add)
                nc.scalar.dma_start(
                    out_c.ap()[128 * off:128 * (off + 1), :], orow[:])

    nc.compile()
    return nc


def make_in_maps(cfg, inputs):
    N, D, QH, FC = cfg['N'], cfg['D'], cfg['QH'], cfg['FC']
    C = CORES
    NB = N // C
    FS = D // C
    bf = ml_dtypes.bfloat16
    f32 = np.float32

    x = np.ascontiguousarray(inputs['x'], dtype=f32)
    anw = np.asarray(inputs['attn_norm_w'], dtype=f32)
    fnw = np.asarray(inputs['ffn_norm_w'], dtype=f32)
    wq = np.asarray(inputs['wq'], dtype=f32) * anw[None, :]
    wk = np.asarray(inputs['wk'], dtype=f32) * anw[None, :]
    wv = np.asarray(inputs['wv'], dtype=f32) * anw[None, :]
    wo = np.asarray(inputs['wo'], dtype=f32)
    wg = np.asarray(inputs['wg'], dtype=f32) * fnw[None, :]
    wh = np.asarray(inputs['wh'], dtype=f32) * fnw[None, :]
    wf = np.asarray(inputs['wf'], dtype=f32)
    rcosT = np.ascontiguousarray(np.asarray(inputs['r_cos'], dtype=f32).T)
    rsinT = np.ascontiguousarray(np.asarray(inputs['r_sin'], dtype=f32).T)

    DH_ = 128
    P = np.zeros((DH_, DH_), dtype=f32)
    for i in range(DH_ // 2):
        P[2 * i, 2 * i + 1] = -1.0
        P[2 * i + 1, 2 * i] = 1.0
    swapT = np.ascontiguousarray(P.T)

    diagneg = np.diag(np.full(DH_, NEG_BIG, dtype=f32))
    ident = np.eye(128, dtype=f32)
    ones = np.ones((128, 128), dtype=f32)
    onesf = np.ones((1, 128), dtype=f32)
    m4 = np.zeros((4, 128, TBLK), dtype=f32)
    for ri in range(4):
        kk = np.arange(128)[:, None] + 128 * ri
        qq = np.arange(TBLK)[None, :]
        m4[ri] = (kk > qq).astype(f32)

    in_maps = []
    for c in range(C):
        qh_rows = slice(QH * DH_ * c, QH * DH_ * (c + 1))
        kv_rows = slice(DH_ * c, DH_ * (c + 1))
        fc_rows = slice(FC * c, FC * (c + 1))
        fs_cols = slice(FS * c, FS * (c + 1))
        in_maps.append({
            "xT_own": np.ascontiguousarray(x[NB * c:NB * (c + 1), :].T),
            "xT_fs": np.ascontiguousarray(x[:, fs_cols].T),
            "wqT": np.ascontiguousarray(wq[qh_rows, :].T).astype(bf),
            "wkT": np.ascontiguousarray(wk[kv_rows, :].T).astype(bf),
            "wvT": np.ascontiguousarray(wv[kv_rows, :].T).astype(bf),
            "woTc": np.ascontiguousarray(wo[fs_cols, :].T).astype(bf),
            "wgT": np.ascontiguousarray(wg[fc_rows, :].T).astype(bf),
            "whT": np.ascontiguousarray(wh[fc_rows, :].T).astype(bf),
            "wfT": np.ascontiguousarray(wf[:, fc_rows].T).astype(bf),
            "rcosT": rcosT.astype(bf),
            "rsinT": rsinT.astype(bf),
            "swapT": swapT.astype(bf),
            "diagneg": diagneg.astype(bf),
            "identb": ident.astype(bf),
            "onesc": ones.astype(bf),
            "onesf": onesf,
            "masks": m4.astype(bf),
        })
    return in_maps


def assemble(results):
    full = np.concatenate([r["out_c"] for r in results], axis=0)  # [D, N]
    return np.ascontiguousarray(full.T)


_NC_CACHE = {}


def get_module(cfg_key=None):
    cfg = FULL_CFG if cfg_key is None else cfg_key
    key = tuple(sorted(cfg.items()))
    if key not in _NC_CACHE:
        _NC_CACHE[key] = build_module(cfg)
    return _NC_CACHE[key]


def run(inputs, cfg=None, trace=False):
    cfg = cfg or FULL_CFG
    nc = get_module(cfg)
    in_maps = make_in_maps(cfg, inputs)
    r = run_bass_kernel_spmd(nc, in_maps, list(range(CORES)), trace=trace)
    return assemble(r.results), r


def kernel(**inputs):
    out, _ = run(inputs)
    return np.asarray(out, dtype=np.float32)
